# revision 1
# baseline (speedup 1.0000x reference)
"""Trainium2 Bass kernel for nn_DecoderAttention (Bahdanau attention + LSTM decoder).

Data-parallel over batch: B=128 split across 8 NeuronCores (16 batches/core).
All FLOPs run on device; the host only reshuffles layouts (transpose / dtype
cast / weight concat with bias rows folded in as an extra contraction row).

Per-core device pipeline:
  phase 0: load weights, build identities, qprojT = Wa @ q^T (+ ba + bua) on PE
  phase 1: per batch b: kprojT = Ua @ enc_b^T on PE (bf16 in, fp32 PSUM),
           tanh(kprojT + qprojT[:, b]) on ACT -> e tiles,
           scores = Va . e on PE (M=1) accumulated into one [16, 2048] PSUM tile
  phase 2: batched softmax over [16, 2048]: reduce_max (DVE), Exp with
           accum_out=Z (ACT); normalization deferred to the context stage
  phase 3: transpose p -> pT chunks [t, b] on PE
  phase 4: context_b = sum_t p[t] enc_b[t, :] on PE (pT stationary), scale by 1/Z
  phase 5: G0 = ctx @ W_ihc^T + q @ W_hh^T + (b_ih + b_hh) on PE (ones-row bias)
  phase 6: 5 serial decoder steps: gates = G0 + x * w_x, LSTM cell, 3-layer MLP
           (MLP matmuls in feature-major layout, biases via ones-row aug)
"""

import numpy as np
import ml_dtypes

B, T, H = 128, 2048, 200
NCORES = 8
NB = B // NCORES  # 16 batches per core
NSTEPS = 5
G4 = 4 * H  # 800 gate width

_CACHE = {}

BF16 = ml_dtypes.bfloat16


def _build_module():
    """Build the Bass module (same NEFF for all 8 cores)."""
    from contextlib import ExitStack

    import concourse.bass as bass
    import concourse.tile as tile
    from concourse import bacc, mybir
    from concourse.masks import make_identity

    dt = mybir.dt
    AF = mybir.ActivationFunctionType
    OP = mybir.AluOpType
    AX = mybir.AxisListType

    nc = bacc.Bacc(
        "TRN2",
        target_bir_lowering=False,
        debug=False,
        num_devices=NCORES,
    )

    # ---- DRAM tensors (per-core shards; weights replicated) ----
    d_encT = nc.dram_tensor("encT", [NB, H, T], dt.bfloat16, kind="ExternalInput").ap()
    d_encN = nc.dram_tensor("encN", [NB, T, H], dt.bfloat16, kind="ExternalInput").ap()
    d_qT = nc.dram_tensor("qT", [H, NB], dt.bfloat16, kind="ExternalInput").ap()
    d_c0 = nc.dram_tensor("c0s", [NB, H], dt.float32, kind="ExternalInput").ap()
    d_x0 = nc.dram_tensor("x0s", [NB, 1], dt.float32, kind="ExternalInput").ap()
    d_UaT = nc.dram_tensor("UaT", [H, H], dt.bfloat16, kind="ExternalInput").ap()
    d_WaT = nc.dram_tensor("WaT", [H, H], dt.bfloat16, kind="ExternalInput").ap()
    d_qb = nc.dram_tensor("qb", [H, 1], dt.float32, kind="ExternalInput").ap()
    d_VaT = nc.dram_tensor("VaT", [H, 1], dt.bfloat16, kind="ExternalInput").ap()
    d_WihcT = nc.dram_tensor(
        "WihcT", [H + 1, G4], dt.bfloat16, kind="ExternalInput"
    ).ap()
    d_WhhT = nc.dram_tensor("WhhT", [H, G4], dt.bfloat16, kind="ExternalInput").ap()
    d_wxr = nc.dram_tensor("wxr", [NB, G4], dt.bfloat16, kind="ExternalInput").ap()
    d_W1T = nc.dram_tensor("W1T", [H + 1, 100], dt.float32, kind="ExternalInput").ap()
    d_W2T = nc.dram_tensor("W2T", [101, 50], dt.float32, kind="ExternalInput").ap()
    d_W3T = nc.dram_tensor("W3T", [51, 1], dt.float32, kind="ExternalInput").ap()
    # ones rows for the bias-row (aug) trick; DMA'd because compute engines
    # cannot write at non-32-aligned partition offsets
    d_ones_f = nc.dram_tensor("ones_f", [1, NB], dt.float32, kind="ExternalInput").ap()
    d_ones_b = nc.dram_tensor("ones_b", [1, NB], dt.bfloat16, kind="ExternalInput").ap()
    d_y = nc.dram_tensor("y", [NSTEPS, NB], dt.float32, kind="ExternalOutput").ap()

    H0, H1 = 128, H - 128  # 128 + 72 partition chunks of the hidden dim
    NT512 = T // 512  # 4

    with tile.TileContext(nc) as tc, ExitStack() as ctx:
        # ---------- persistent pools ----------
        wpool = ctx.enter_context(tc.tile_pool(name="weights", bufs=1))
        spool = ctx.enter_context(tc.tile_pool(name="smalls", bufs=1))

        # warmup: preload the ACT table set (tanh/exp) while DMAs stream
        wt_a = spool.tile([1, 8], dt.float32)
        nc.vector.memset(wt_a[:], 0.0)
        wt_b = spool.tile([1, 8], dt.float32)
        nc.scalar.activation(wt_b[:], wt_a[:], AF.Tanh)

        # identities for PE transposes
        id_bf = wpool.tile([128, 128], dt.bfloat16)
        make_identity(nc, id_bf[:])
        id_f32 = wpool.tile([128, 128], dt.float32)
        make_identity(nc, id_f32[:])

        # attention weights
        ua0 = wpool.tile([H0, H], dt.bfloat16)
        ua1 = wpool.tile([H1, H], dt.bfloat16)
        wa0 = wpool.tile([H0, H], dt.bfloat16)
        wa1 = wpool.tile([H1, H], dt.bfloat16)
        nc.scalar.dma_start(wa0[:], d_WaT[0:H0, :])
        nc.scalar.dma_start(wa1[:], d_WaT[H0:H, :])
        va0 = wpool.tile([H0, 1], dt.bfloat16)
        va1 = wpool.tile([H1, 1], dt.bfloat16)
        nc.scalar.dma_start(va0[:], d_VaT[0:H0, :])
        nc.scalar.dma_start(va1[:], d_VaT[H0:H, :])
        qt0 = wpool.tile([H0, NB], dt.bfloat16)
        qt1 = wpool.tile([H1, NB], dt.bfloat16)
        nc.scalar.dma_start(qt0[:], d_qT[0:H0, :])
        nc.scalar.dma_start(qt1[:], d_qT[H0:H, :])
        qb0 = wpool.tile([H0, 1], dt.float32)
        qb1 = wpool.tile([H1, 1], dt.float32)
        nc.scalar.dma_start(qb0[:], d_qb[0:H0, :])
        nc.scalar.dma_start(qb1[:], d_qb[H0:H, :])

        # decoder weights (allocated now, DMA'd later to keep the SP DGE ring
        # clear for encT during the attention phase)
        wihc0 = wpool.tile([128, G4], dt.bfloat16)
        wihc1 = wpool.tile([73, G4], dt.bfloat16)
        whh0 = wpool.tile([H0, G4], dt.bfloat16)
        whh1 = wpool.tile([H1, G4], dt.bfloat16)
        wxr_sb = wpool.tile([NB, G4], dt.bfloat16)
        w1t0 = wpool.tile([128, 100], dt.float32)
        w1t1 = wpool.tile([73, 100], dt.float32)
        w2t = wpool.tile([101, 50], dt.float32)
        w3t = wpool.tile([51, 1], dt.float32)
        c0_sb = spool.tile([NB, H], dt.float32)

        # ---------- phase 0: qprojT = Wa @ q^T + (ba + bua) ----------
        # out[h, b] = sum_h' WaT[h', h] * qT[h', b]
        qproj0 = spool.tile([H0, NB], dt.float32)  # fp32 bias tiles for tanh
        qproj1 = spool.tile([H1, NB], dt.float32)
        with tc.tile_pool(name="qp_psum", bufs=1, space="PSUM") as qp_ps:
            for m, (msz, qdst, qb) in enumerate(
                [(H0, qproj0, qb0), (H1, qproj1, qb1)]
            ):
                mlo = m * H0
                ps = qp_ps.tile([128, NB], dt.float32, tag="qp")
                nc.tensor.matmul(
                    ps[0:msz, :], wa0[:, mlo : mlo + msz], qt0[:], start=True, stop=False
                )
                nc.tensor.matmul(
                    ps[0:msz, :], wa1[:, mlo : mlo + msz], qt1[:], start=False, stop=True
                )
                # qproj += (ba + bua), per-partition scalar on DVE (keeps ACT
                # free until the first tanh)
                nc.vector.tensor_scalar_add(qdst[:], ps[0:msz, :], qb[:])

        # ---------- phase 1: kproj + tanh + scores + exp ----------
        # Scores land on PSUM partitions {0,32,64,96} (col-tiled waves of 4
        # batches). No max-subtraction: |scores| <= ||Va||_1 ~ 8, exp cannot
        # overflow fp32, so exp + accum_out run directly on the strided tile.
        NW = NB // 4  # 4 waves of 4 batches
        p_str = []  # per-wave exp(scores), batch rows at partition 32j
        z_str = []  # per-wave row sums (Z) at partition 32j
        for w in range(NW):
            p_str.append(spool.tile([128, T], dt.bfloat16, name=f"p_str{w}", tag=f"p_str{w}"))
            z_str.append(spool.tile([128, 1], dt.float32, name=f"z_str{w}", tag=f"z_str{w}"))
        # SBUF pools span phases 1-4 so the scheduler can prefetch encN DMAs
        # during the kproj/tanh phase
        encT_pool = ctx.enter_context(tc.tile_pool(name="encT_pool", bufs=3))

        e_pool = ctx.enter_context(tc.tile_pool(name="e_pool", bufs=4))
        encN_pool = ctx.enter_context(tc.tile_pool(name="encN_pool", bufs=10))
        en_tiles = []
        with (
            tc.tile_pool(name="kp_psum", bufs=2, space="PSUM") as kp_ps,
            tc.tile_pool(name="sc_psum", bufs=1, space="PSUM") as sc_ps,
        ):
            sc = None
            for b in range(NB):
                if b % 4 == 0:
                    # one PSUM tile per wave of 4 batches; batch j of the wave
                    # writes partition 32*j via col-tiling (PE out base must be
                    # 32-aligned)
                    sc = sc_ps.tile([128, T], dt.float32, tag="sc")
                et0 = encT_pool.tile([H0, T], dt.bfloat16, tag="et0")
                et1 = encT_pool.tile([H1, T], dt.bfloat16, tag="et1")
                nc.sync.dma_start(et0[:], d_encT[b, 0:H0, :])
                i_et1 = nc.sync.dma_start(et1[:], d_encT[b, H0:H, :])
                if b == 0:
                    # Ua right behind the first encT pair on the SP ring: the
                    # first kproj waits on encT, not on Ua
                    nc.sync.dma_start(ua0[:], d_UaT[0:H0, :])
                    nc.sync.dma_start(ua1[:], d_UaT[H0:H, :])
                e0 = e_pool.tile([H0, T], dt.bfloat16, tag="e0")
                e1 = e_pool.tile([H1, T], dt.bfloat16, tag="e1")
                for m, (msz, edst, qp) in enumerate(
                    [(H0, e0, qproj0), (H1, e1, qproj1)]
                ):
                    mlo = m * H0
                    for th in range(2):  # 1024-wide psum tiles
                        ps = kp_ps.tile([128, 1024], dt.float32, tag="kp")
                        for n in range(2):
                            c0c = th * 1024 + n * 512
                            nc.tensor.matmul(
                                ps[0:msz, n * 512 : (n + 1) * 512],
                                ua0[:, mlo : mlo + msz],
                                et0[:, c0c : c0c + 512],
                                start=True,
                                stop=False,
                            )
                            i_kp = nc.tensor.matmul(
                                ps[0:msz, n * 512 : (n + 1) * 512],
                                ua1[:, mlo : mlo + msz],
                                et1[:, c0c : c0c + 512],
                                start=False,
                                stop=True,
                            )
                        # e = tanh(kproj + qproj[:, b]) ; write bf16
                        nc.scalar.activation(
                            edst[:, th * 1024 : (th + 1) * 1024],
                            ps[0:msz, :],
                            AF.Tanh,
                            bias=qp[:, b : b + 1],
                        )
                # scores[b, t] = sum_h Va[h] e[h, t]
                j = b % 4
                for t5 in range(NT512):
                    tlo = t5 * 512
                    nc.tensor.matmul(
                        sc[32 * j : 32 * j + 1, tlo : tlo + 512],
                        va0[:],
                        e0[:, tlo : tlo + 512],
                        start=True,
                        stop=False,
                        tile_position=(0, 32 * j),
                    )
                    nc.tensor.matmul(
                        sc[32 * j : 32 * j + 1, tlo : tlo + 512],
                        va1[:],
                        e1[:, tlo : tlo + 512],
                        start=False,
                        stop=True,
                        tile_position=(0, 32 * j),
                    )
                if b < 10:
                    # encN loads paced on the (otherwise idle) SWDGE ring,
                    # one per attention batch; gated behind this batch's encT
                    # so attention keeps HBM priority
                    import bass_rust as _br
                    en = encN_pool.tile(
                        [128, (T // 128) * H], dt.bfloat16,
                        name=f"en{b}", tag="en",
                    )
                    i_en = nc.gpsimd.dma_start(
                        en[:],
                        d_encN[b].rearrange("(p n) h -> p (n h)", p=128),
                    )
                    _br.add_dep_helper(
                        i_en.ins, i_kp.ins, sync=True,
                        reason="encN paced behind this batch's kproj",
                    )
                    en_tiles.append(en)
                if b % 4 == 3:
                    # p = exp(scores), Z = row sums — two halves so the next
                    # wave's score MMs unblock bank-by-bank
                    w = b // 4
                    za = e_pool.tile([128, 1], dt.float32, tag="za")
                    zb = e_pool.tile([128, 1], dt.float32, tag="zb")
                    nc.scalar.activation(
                        p_str[w][:, 0:1024], sc[:, 0:1024], AF.Exp, accum_out=za[:]
                    )
                    nc.scalar.activation(
                        p_str[w][:, 1024:2048],
                        sc[:, 1024:2048],
                        AF.Exp,
                        accum_out=zb[:],
                    )
                    nc.vector.tensor_tensor(z_str[w][:], za[:], zb[:], op=OP.add)

        # deferred decoder-weight loads (SP ring is now free)
        nc.sync.dma_start(wihc0[:], d_WihcT[0:128, :])
        nc.sync.dma_start(wihc1[:], d_WihcT[128 : H + 1, :])
        nc.sync.dma_start(whh0[:], d_WhhT[0:H0, :])
        nc.sync.dma_start(whh1[:], d_WhhT[H0:H, :])
        nc.sync.dma_start(wxr_sb[:], d_wxr[:, :])
        nc.sync.dma_start(w1t0[:], d_W1T[0:128, :])
        nc.sync.dma_start(w1t1[:], d_W1T[128 : H + 1, :])
        nc.sync.dma_start(w2t[:], d_W2T[:, :])
        nc.sync.dma_start(w3t[:], d_W3T[:, :])
        nc.sync.dma_start(c0_sb[:], d_c0[:, :])

        for bb in range(10, NB):
            en = encN_pool.tile(
                [128, (T // 128) * H], dt.bfloat16, name=f"en{bb}", tag="en"
            )
            nc.gpsimd.dma_start(
                en[:], d_encN[bb].rearrange("(p n) h -> p (n h)", p=128)
            )
            en_tiles.append(en)

        # ---------- phases 3+4 (interleaved per wave): pT + context ----------
        # PE transpose moves the strided batch rows into columns {0,32,64,96};
        # a strided FREE-dim slice is legal, so compact with a DVE copy.
        # pT chunk c = t stride-class (t = 16*k + c), matching the
        # contiguous-per-partition encN layout "(p n) h".
        NCH = T // 128  # 16
        pt_sb = spool.tile([128, NCH * NB], dt.bfloat16)
        ct0 = spool.tile([H0, NB], dt.bfloat16)
        ct1 = spool.tile([H1 + 1, NB], dt.bfloat16)  # row 72 = ones (bias row)
        nc.scalar.dma_start(ct1[H1 : H1 + 1, :], d_ones_b[:, :])
        with (
            tc.tile_pool(name="ctx_psum", bufs=2, space="PSUM") as ctx_ps,
            tc.tile_pool(name="ctx_sb", bufs=2) as ctx_sb_pool,
        ):
            for w in range(NW):
                for c in range(NCH):
                    ps = ctx_ps.tile([128, 128], dt.bfloat16, tag="pt")
                    nc.tensor.transpose(
                        ps[:], p_str[w][:, c : T : 16], id_bf[:]
                    )
                    nc.vector.tensor_copy(
                        pt_sb[:, c * NB + 4 * w : c * NB + 4 * w + 4],
                        ps[:, 0:128:32],
                    )
                cw = ctx_ps.tile([128, H], dt.float32, tag="cw")
                # c-outer / j-inner: adjacent MMs hit disjoint PE col groups,
                # so the 4 batches' context accumulations run concurrently
                for c in range(NCH):
                    for j in range(4):
                        b = 4 * w + j
                        nc.tensor.matmul(
                            cw[32 * j : 32 * j + 1, :],
                            pt_sb[:, c * NB + b : c * NB + b + 1],
                            en_tiles[b][:, c * H : (c + 1) * H],
                            start=(c == 0),
                            stop=(c == NCH - 1),
                            tile_position=(0, 32 * j),
                        )
                # normalize by 1/Z in the strided layout, cast to bf16
                rz = ctx_sb_pool.tile([128, 1], dt.float32, tag="rz")
                nc.vector.reciprocal(rz[:], z_str[w][:])
                cs = ctx_sb_pool.tile([128, H], dt.bfloat16, tag="cs")
                nc.vector.tensor_scalar_mul(cs[:], cw[:], rz[:, 0:1])
                # transpose into ctxT columns 4w..4w+3
                tp0 = ctx_ps.tile([128, 128], dt.bfloat16, tag="ctp")
                nc.tensor.transpose(tp0[:], cs[:, 0:H0], id_bf[:])
                nc.vector.tensor_copy(ct0[:, 4 * w : 4 * w + 4], tp0[:, 0:128:32])
                tp1 = ctx_ps.tile([128, 128], dt.bfloat16, tag="ctp")
                nc.tensor.transpose(tp1[0:H1, :], cs[:, H0:H], id_bf[:])
                nc.vector.tensor_copy(
                    ct1[0:H1, 4 * w : 4 * w + 4], tp1[0:H1, 0:128:32]
                )

        g0_bf = spool.tile([NB, G4], dt.bfloat16)
        with tc.tile_pool(name="g0_psum", bufs=1, space="PSUM") as g0_ps:
            gp = g0_ps.tile([NB, G4], dt.float32, tag="g0")
            for n, nsz in [(0, 512), (512, G4 - 512)]:
                nc.tensor.matmul(
                    gp[:, n : n + nsz],
                    ct0[:],
                    wihc0[:, n : n + nsz],
                    start=True,
                    stop=False,
                )
                nc.tensor.matmul(
                    gp[:, n : n + nsz],
                    ct1[:],
                    wihc1[:, n : n + nsz],
                    start=False,
                    stop=False,
                )
                nc.tensor.matmul(
                    gp[:, n : n + nsz],
                    qt0[:],
                    whh0[:, n : n + nsz],
                    start=False,
                    stop=False,
                )
                nc.tensor.matmul(
                    gp[:, n : n + nsz],
                    qt1[:],
                    whh1[:, n : n + nsz],
                    start=False,
                    stop=True,
                )
            nc.vector.tensor_copy(g0_bf[:], gp[:])

        # ---------- phase 6: decoder steps ----------
        ht0 = spool.tile([H0, NB], dt.float32)
        ht1 = spool.tile([H1 + 1, NB], dt.float32)  # row 72 = ones (b1 row)
        nc.sync.dma_start(ht1[H1 : H1 + 1, :], d_ones_f[:, :])
        o1t = spool.tile([101, NB], dt.float32)  # row 100 = ones (b2 row)
        nc.sync.dma_start(o1t[100:101, :], d_ones_f[:, :])
        o2t = spool.tile([51, NB], dt.float32)  # row 50 = ones (b3 row)
        nc.sync.dma_start(o2t[50:51, :], d_ones_f[:, :])

        x_sb = spool.tile([NB, 1], dt.float32)
        nc.sync.dma_start(x_sb[:], d_x0[:, :])

        with (
            tc.tile_pool(name="ls", bufs=2) as ls,
            tc.tile_pool(name="ls_psum", bufs=3, space="PSUM") as lp,
        ):
            xt = x_sb
            for t in range(NSTEPS):
                gates = ls.tile([NB, G4], dt.bfloat16, tag="gates")
                # gates = wxr * x (per-partition scalar)
                nc.vector.tensor_scalar_mul(gates[:], wxr_sb[:], xt[:, 0:1])
                gates2 = ls.tile([NB, G4], dt.bfloat16, tag="gates2")
                nc.vector.tensor_tensor(gates2[:], gates[:], g0_bf[:], op=OP.add)
                # per-gate activations, forget-gate first so the DVE chain
                # starts as early as possible
                sf = ls.tile([NB, H], dt.float32, tag="sf")
                nc.scalar.activation(sf[:], gates2[:, H : 2 * H], AF.Sigmoid)
                g2 = ls.tile([NB, H], dt.float32, tag="g2")
                nc.scalar.activation(g2[:], gates2[:, 2 * H : 3 * H], AF.Tanh)
                si = ls.tile([NB, H], dt.float32, tag="si")
                nc.scalar.activation(si[:], gates2[:, 0:H], AF.Sigmoid)
                so = ls.tile([NB, H], dt.float32, tag="so")
                nc.scalar.activation(so[:], gates2[:, 3 * H : 4 * H], AF.Sigmoid)
                t1 = ls.tile([NB, H], dt.float32, tag="t1")
                nc.vector.tensor_tensor(t1[:], sf[:], c0_sb[:], op=OP.mult)
                t2 = ls.tile([NB, H], dt.float32, tag="t2")
                nc.vector.tensor_tensor(t2[:], si[:], g2[:], op=OP.mult)
                cn = ls.tile([NB, H], dt.float32, tag="cn")
                nc.vector.tensor_tensor(cn[:], t1[:], t2[:], op=OP.add)
                tcn = ls.tile([NB, H], dt.float32, tag="tcn")
                nc.scalar.activation(tcn[:], cn[:], AF.Tanh)
                hh = ls.tile([NB, H], dt.float32, tag="hh")
                nc.vector.tensor_tensor(hh[:], so[:], tcn[:], op=OP.mult)
                # transpose h -> ht0/ht1 (feature-major for the MLP); relu
                # commutes with transpose, so fold it into the PSUM copies
                tp0 = lp.tile([128, NB], dt.float32, tag="lsps")
                nc.tensor.transpose(tp0[:], hh[:, 0:H0], id_f32[0:NB, 0:NB])
                nc.scalar.activation(ht0[:], tp0[:], AF.Relu)
                tp1 = lp.tile([128, NB], dt.float32, tag="lsps")
                nc.tensor.transpose(tp1[0:H1, :], hh[:, H0:H], id_f32[0:NB, 0:NB])
                nc.scalar.activation(ht1[0:H1, :], tp1[0:H1, :], AF.Relu)
                # MLP: out1 = relu(W1 @ h + b1) in feature-major
                m1 = lp.tile([100, NB], dt.float32, tag="lsps")
                nc.tensor.matmul(m1[:], w1t0[:], ht0[:], start=True, stop=False)
                nc.tensor.matmul(m1[:], w1t1[:], ht1[:], start=False, stop=True)
                nc.scalar.activation(o1t[0:100, :], m1[:], AF.Relu)
                m2 = lp.tile([50, NB], dt.float32, tag="lsps")
                nc.tensor.matmul(m2[:], w2t[:], o1t[:], start=True, stop=True)
                nc.scalar.activation(o2t[0:50, :], m2[:], AF.Relu)
                m3 = lp.tile([1, NB], dt.float32, tag="lsps")
                nc.tensor.matmul(m3[:], w3t[:], o2t[:], start=True, stop=True)
                y_row = ls.tile([1, NB], dt.float32, tag="y_row")
                nc.vector.tensor_copy(y_row[:], m3[:])
                nc.sync.dma_start(d_y[t : t + 1, :], y_row[:])
                if t < NSTEPS - 1:
                    # y row back to [b, 1] for the next step's scalar multiply
                    xp = lp.tile([NB, 1], dt.float32, tag="lsps")
                    nc.tensor.transpose(xp[:], y_row[:], id_f32[0:1, 0:1])
                    xn = ls.tile([NB, 1], dt.float32, tag="xn")
                    nc.scalar.copy(xn[:], xp[:])
                    xt = xn

    # Bacc lowering: register allocation + wait splitting (<=1 wait/inst on HW)
    nc.compile()
    return nc


def _prep_inputs(x, h0, c0, encoder_output, Wa, ba, Ua, bua, Va, bva,
                 W_ih, W_hh, b_ih, b_hh, W1, b1, W2, b2, W3, b3):
    """Host-side layout prep -> list of per-core input maps."""
    f32 = np.float32
    enc = np.ascontiguousarray(encoder_output, dtype=f32)
    q = np.asarray(h0, dtype=f32)[0]          # [B, H]
    c0f = np.asarray(c0, dtype=f32)[0]        # [B, H]
    x0 = np.asarray(x, dtype=f32).reshape(B, 1)

    # replicated weights (shared by every core)
    shared = {
        "UaT": np.ascontiguousarray(np.asarray(Ua, f32).T).astype(BF16),
        "WaT": np.ascontiguousarray(np.asarray(Wa, f32).T).astype(BF16),
        "qb": (np.asarray(ba, f32) + np.asarray(bua, f32)).reshape(H, 1),
        "VaT": np.ascontiguousarray(np.asarray(Va, f32)[0].reshape(H, 1)).astype(BF16),
        "WihcT": np.concatenate(
            [
                np.asarray(W_ih, f32)[:, 1:].T,
                (np.asarray(b_ih, f32) + np.asarray(b_hh, f32)).reshape(1, G4),
            ],
            axis=0,
        ).astype(BF16),
        "WhhT": np.ascontiguousarray(np.asarray(W_hh, f32).T).astype(BF16),
        "wxr": np.broadcast_to(
            np.asarray(W_ih, f32)[:, 0].reshape(1, G4), (NB, G4)
        ).astype(BF16),
        "W1T": np.concatenate(
            [np.asarray(W1, f32).T, np.asarray(b1, f32).reshape(1, 100)], axis=0
        ),
        "W2T": np.concatenate(
            [np.asarray(W2, f32).T, np.asarray(b2, f32).reshape(1, 50)], axis=0
        ),
        "W3T": np.concatenate(
            [np.asarray(W3, f32).T, np.asarray(b3, f32).reshape(1, 1)], axis=0
        ),
        "ones_f": np.ones((1, NB), f32),
        "ones_b": np.ones((1, NB), BF16),
    }

    in_maps = []
    for c in range(NCORES):
        bs = slice(c * NB, (c + 1) * NB)
        enc_c = enc[bs]  # [NB, T, H]
        m = dict(shared)
        m["encT"] = np.ascontiguousarray(enc_c.transpose(0, 2, 1)).astype(BF16)
        m["encN"] = enc_c.astype(BF16)
        m["qT"] = np.ascontiguousarray(q[bs].T).astype(BF16)
        m["c0s"] = np.ascontiguousarray(c0f[bs])
        m["x0s"] = np.ascontiguousarray(x0[bs])
        in_maps.append(m)
    return in_maps


def kernel(**inputs):
    from concourse.bass_utils import run_bass_kernel_spmd

    if "nc" not in _CACHE:
        _CACHE["nc"] = _build_module()
    nc = _CACHE["nc"]

    in_maps = _prep_inputs(**inputs)
    res = run_bass_kernel_spmd(nc, in_maps, core_ids=list(range(NCORES)))
    # y per core: [NSTEPS, NB] -> full output [B, NSTEPS]
    out = np.concatenate([r["y"].T for r in res.results], axis=0)
    return np.ascontiguousarray(out.astype(np.float32))



# revision 4
# speedup vs baseline: 1.3289x; 1.3289x over previous
"""Trainium2 Bass kernel for nn_DecoderAttention (Bahdanau attention + LSTM decoder).

Data-parallel over batch: B=128 split across 8 NeuronCores (16 batches/core).
All FLOPs run on device; the host only reshuffles layouts (transpose / dtype
cast / weight concat with bias rows folded in as an extra contraction row).

Per-core device pipeline (cost-model-aware layout):
  phase 0: load weights, identity, qprojT = Wa @ q^T (+ ba + bua) on PE
  phase 1: per batch b: kprojT = Ua @ enc_b^T on PE (bf16 in, fp32 PSUM),
           tanh(kprojT + qprojT[:, b]) on ACT -> e tiles [h, t] in SBUF;
           scores via FLIPPED matmuls: e chunk stationary (K=h, M=128 t's of
           one stride class), Va moving (N=1) -> scores land [t, (b,chunk)]
           columns of ONE PSUM tile; N=1 makes these matmuls ~free and kills
           the pT transpose phase entirely.
  phase 2: ONE exp over the [128, 256] scores tile -> p (bf16, unnormalized);
           Z per batch via two tiny PE reductions (colsum then chunk-sum),
           broadcast 1/Z to [128, 16] via a K=1 outer-product matmul.
  phase 3: context via FLIPPED matmuls: encN chunk stationary (K=t, M=h),
           p column moving (N=1) -> ctxT [h, b] accumulates directly in the
           layout G0 needs (no transposes); scale by 1/Z on DVE.
  phase 4: G0 = ctx @ W_ihc^T + q @ W_hh^T + (b_ih + b_hh) on PE
  phase 5: 5 serial decoder steps, all-bf16 elementwise:
           gates = G0 + x*wxr fused on DVE (scalar_tensor_tensor),
           gate order permuted to (f,i,o | g) so one sigmoid covers f,i,o,
           MLP in bf16 with m3 operands swapped so x_next = out [16, 1].
"""

import numpy as np
import ml_dtypes

B, T, H = 128, 2048, 200
NCORES = 8
NB = B // NCORES  # 16 batches per core
NSTEPS = 5
G4 = 4 * H  # 800 gate width
NCH = T // 128  # 16 stride-class chunks (t = 16*k + c -> partition k, chunk c)

_CACHE = {}

BF16 = ml_dtypes.bfloat16


def _build_module():
    """Build the Bass module (same NEFF for all 8 cores)."""
    from contextlib import ExitStack

    import concourse.bass as bass
    import concourse.tile as tile
    from concourse import bacc, mybir
    from concourse.masks import make_identity

    dt = mybir.dt
    AF = mybir.ActivationFunctionType
    OP = mybir.AluOpType

    nc = bacc.Bacc(
        "TRN2",
        target_bir_lowering=False,
        debug=False,
        num_devices=NCORES,
    )

    # ---- DRAM tensors (per-core shards; weights replicated) ----
    d_encT = nc.dram_tensor("encT", [NB, H, T], dt.bfloat16, kind="ExternalInput").ap()
    d_encN = nc.dram_tensor("encN", [NB, T, H], dt.bfloat16, kind="ExternalInput").ap()
    d_qT = nc.dram_tensor("qT", [H, NB], dt.bfloat16, kind="ExternalInput").ap()
    d_c0 = nc.dram_tensor("c0s", [NB, H], dt.bfloat16, kind="ExternalInput").ap()
    d_x0 = nc.dram_tensor("x0s", [NB, 1], dt.float32, kind="ExternalInput").ap()
    d_UaT = nc.dram_tensor("UaT", [H, H], dt.bfloat16, kind="ExternalInput").ap()
    d_WaT = nc.dram_tensor("WaT", [H, H], dt.bfloat16, kind="ExternalInput").ap()
    d_qb = nc.dram_tensor("qb", [H, 1], dt.float32, kind="ExternalInput").ap()
    d_VaT = nc.dram_tensor("VaT", [H, 1], dt.bfloat16, kind="ExternalInput").ap()
    d_WihcT = nc.dram_tensor(
        "WihcT", [H + 1, G4], dt.bfloat16, kind="ExternalInput"
    ).ap()
    d_WhhT = nc.dram_tensor("WhhT", [H, G4], dt.bfloat16, kind="ExternalInput").ap()
    d_wxr = nc.dram_tensor("wxr", [NB, G4], dt.bfloat16, kind="ExternalInput").ap()
    d_W1T = nc.dram_tensor("W1T", [H + 1, 100], dt.bfloat16, kind="ExternalInput").ap()
    d_W2T = nc.dram_tensor("W2T", [101, 50], dt.bfloat16, kind="ExternalInput").ap()
    d_W3T = nc.dram_tensor("W3T", [51, 1], dt.bfloat16, kind="ExternalInput").ap()
    # ones rows for the bias-row (aug) trick; DMA'd because compute engines
    # cannot write at non-32-aligned partition offsets
    d_ones_b = nc.dram_tensor("ones_b", [1, NB], dt.bfloat16, kind="ExternalInput").ap()
    d_y = nc.dram_tensor("y2", [NB, NSTEPS], dt.float32, kind="ExternalOutput").ap()

    H0, H1 = 128, H - 128  # 128 + 72 partition chunks of the hidden dim

    with tile.TileContext(nc) as tc, ExitStack() as ctx:
        # ---------- persistent pools ----------
        wpool = ctx.enter_context(tc.tile_pool(name="weights", bufs=1))
        spool = ctx.enter_context(tc.tile_pool(name="smalls", bufs=1))

        # warmup: preload the ACT table sets (tanh/exp + sigmoid) while DMAs
        # stream, so no table load lands mid-kernel
        wt_a = spool.tile([1, 8], dt.float32)
        nc.vector.memset(wt_a[:], 0.0)
        wt_b = spool.tile([1, 8], dt.float32)
        nc.scalar.activation(wt_b[:], wt_a[:], AF.Tanh)
        nc.scalar.activation(wt_b[:], wt_a[:], AF.Sigmoid)
        nc.scalar.activation(wt_b[:], wt_a[:], AF.Exp)

        # identity for the decoder's h transposes (bf16)
        id_bf = wpool.tile([128, 128], dt.bfloat16)
        make_identity(nc, id_bf[:])

        # ones columns/rows for the tiny PE reductions (sliced on read; memset
        # writes full 128-partition tiles so partition offsets stay aligned)
        ones_c_bf = wpool.tile([128, 1], dt.bfloat16)
        nc.vector.memset(ones_c_bf[:], 1.0)
        ones_c_f = wpool.tile([128, 1], dt.float32)
        nc.vector.memset(ones_c_f[:], 1.0)
        ones_r_f = wpool.tile([1, 128], dt.float32)
        nc.vector.memset(ones_r_f[:], 1.0)

        # attention weights
        ua0 = wpool.tile([H0, H], dt.bfloat16)
        ua1 = wpool.tile([H1, H], dt.bfloat16)
        wa0 = wpool.tile([H0, H], dt.bfloat16)
        wa1 = wpool.tile([H1, H], dt.bfloat16)
        nc.scalar.dma_start(wa0[:], d_WaT[0:H0, :])
        nc.scalar.dma_start(wa1[:], d_WaT[H0:H, :])
        va0 = wpool.tile([H0, 1], dt.bfloat16)
        va1 = wpool.tile([H1, 1], dt.bfloat16)
        nc.scalar.dma_start(va0[:], d_VaT[0:H0, :])
        nc.scalar.dma_start(va1[:], d_VaT[H0:H, :])
        qt0 = wpool.tile([H0, NB], dt.bfloat16)
        qt1 = wpool.tile([H1, NB], dt.bfloat16)
        nc.scalar.dma_start(qt0[:], d_qT[0:H0, :])
        nc.scalar.dma_start(qt1[:], d_qT[H0:H, :])
        qb0 = wpool.tile([H0, 1], dt.float32)
        qb1 = wpool.tile([H1, 1], dt.float32)
        nc.scalar.dma_start(qb0[:], d_qb[0:H0, :])
        nc.scalar.dma_start(qb1[:], d_qb[H0:H, :])

        # decoder weights (allocated now, DMA'd later to keep the SP DGE ring
        # clear for encT during the attention phase)
        wihc0 = wpool.tile([128, G4], dt.bfloat16)
        wihc1 = wpool.tile([73, G4], dt.bfloat16)
        whh0 = wpool.tile([H0, G4], dt.bfloat16)
        whh1 = wpool.tile([H1, G4], dt.bfloat16)
        wxr_sb = wpool.tile([NB, G4], dt.bfloat16)
        w1t0 = wpool.tile([128, 100], dt.bfloat16)
        w1t1 = wpool.tile([73, 100], dt.bfloat16)
        w2t = wpool.tile([101, 50], dt.bfloat16)
        w3t = wpool.tile([51, 1], dt.bfloat16)
        c0_sb = spool.tile([NB, H], dt.bfloat16)

        # ---------- phase 0: qprojT = Wa @ q^T + (ba + bua) ----------
        qproj0 = spool.tile([H0, NB], dt.float32)  # fp32 bias tiles for tanh
        qproj1 = spool.tile([H1, NB], dt.float32)
        with tc.tile_pool(name="qp_psum", bufs=1, space="PSUM") as qp_ps:
            for m, (msz, qdst, qb) in enumerate(
                [(H0, qproj0, qb0), (H1, qproj1, qb1)]
            ):
                mlo = m * H0
                ps = qp_ps.tile([128, NB], dt.float32, tag="qp")
                nc.tensor.matmul(
                    ps[0:msz, :], wa0[:, mlo : mlo + msz], qt0[:], start=True, stop=False
                )
                nc.tensor.matmul(
                    ps[0:msz, :], wa1[:, mlo : mlo + msz], qt1[:], start=False, stop=True
                )
                nc.vector.tensor_scalar_add(qdst[:], ps[0:msz, :], qb[:])

        # ---------- phase 1: kproj + tanh + scores (flipped) ----------
        # scores land in ONE [128, NB*NCH] PSUM tile: column 16*b + c holds
        # the 128 t's of stride class c (t = 16*k + c -> partition k), which
        # matches the encN "(p n) h" SBUF layout used by the context matmuls.
        encT_pool = ctx.enter_context(tc.tile_pool(name="encT_pool", bufs=3))
        e_pool = ctx.enter_context(tc.tile_pool(name="e_pool", bufs=4))
        encN_pool = ctx.enter_context(tc.tile_pool(name="encN_pool", bufs=10))
        sc_pool = ctx.enter_context(tc.tile_pool(name="sc_psum", bufs=1, space="PSUM"))
        sc = sc_pool.tile([128, NB * NCH], dt.float32, tag="sc")
        p_sb = spool.tile([128, NB * NCH], dt.bfloat16)

        en_tiles = []
        e_saved = []  # (e0, e1) per batch, scores emitted with 1-batch skew

        def emit_scores(b, e0, e1):
            for c in range(NCH):
                col = NB * 0 + b * NCH + c
                nc.tensor.matmul(
                    sc[:, col : col + 1],
                    e0[:, c : T : NCH],
                    va0[:],
                    start=True,
                    stop=False,
                )
                nc.tensor.matmul(
                    sc[:, col : col + 1],
                    e1[:, c : T : NCH],
                    va1[:],
                    start=False,
                    stop=True,
                )

        with tc.tile_pool(name="kp_psum", bufs=2, space="PSUM") as kp_ps:
            for b in range(NB):
                et0 = encT_pool.tile([H0, T], dt.bfloat16, tag="et0")
                et1 = encT_pool.tile([H1, T], dt.bfloat16, tag="et1")
                nc.sync.dma_start(et0[:], d_encT[b, 0:H0, :])
                i_et1 = nc.sync.dma_start(et1[:], d_encT[b, H0:H, :])
                if b == 0:
                    # Ua right behind the first encT pair on the SP ring
                    nc.sync.dma_start(ua0[:], d_UaT[0:H0, :])
                    nc.sync.dma_start(ua1[:], d_UaT[H0:H, :])
                e0 = e_pool.tile([H0, T], dt.bfloat16, tag="e0")
                e1 = e_pool.tile([H1, T], dt.bfloat16, tag="e1")
                i_kp = None
                for m, (msz, edst, qp) in enumerate(
                    [(H0, e0, qproj0), (H1, e1, qproj1)]
                ):
                    mlo = m * H0
                    for th in range(2):  # 1024-wide psum tiles
                        ps = kp_ps.tile([128, 1024], dt.float32, tag="kp")
                        for n in range(2):
                            c0c = th * 1024 + n * 512
                            nc.tensor.matmul(
                                ps[0:msz, n * 512 : (n + 1) * 512],
                                ua0[:, mlo : mlo + msz],
                                et0[:, c0c : c0c + 512],
                                start=True,
                                stop=False,
                            )
                            i_kp = nc.tensor.matmul(
                                ps[0:msz, n * 512 : (n + 1) * 512],
                                ua1[:, mlo : mlo + msz],
                                et1[:, c0c : c0c + 512],
                                start=False,
                                stop=True,
                            )
                        # e = tanh(kproj + qproj[:, b]) ; write bf16
                        nc.scalar.activation(
                            edst[:, th * 1024 : (th + 1) * 1024],
                            ps[0:msz, :],
                            AF.Tanh,
                            bias=qp[:, b : b + 1],
                        )
                # scores with a 1-batch skew so the PE never stalls waiting on
                # this batch's tanh while the next batch's kproj is runnable
                if b > 0:
                    emit_scores(b - 1, *e_saved[b - 1])
                e_saved.append((e0, e1))
                if b < 10:
                    # encN loads paced on the (otherwise idle) SWDGE ring,
                    # gated behind this batch's kproj to keep encT priority
                    import bass_rust as _br

                    en = encN_pool.tile(
                        [128, NCH * H], dt.bfloat16, name=f"en{b}", tag="en"
                    )
                    i_en = nc.gpsimd.dma_start(
                        en[:],
                        d_encN[b].rearrange("(p n) h -> p (n h)", p=128),
                    )
                    _br.add_dep_helper(
                        i_en.ins, i_kp.ins, sync=True,
                        reason="encN paced behind this batch's kproj",
                    )
                    en_tiles.append(en)
            emit_scores(NB - 1, *e_saved[NB - 1])

        # deferred decoder-weight loads (SP ring is now free)
        nc.sync.dma_start(wihc0[:], d_WihcT[0:128, :])
        nc.sync.dma_start(wihc1[:], d_WihcT[128 : H + 1, :])
        nc.sync.dma_start(whh0[:], d_WhhT[0:H0, :])
        nc.sync.dma_start(whh1[:], d_WhhT[H0:H, :])
        nc.sync.dma_start(wxr_sb[:], d_wxr[:, :])
        nc.sync.dma_start(w1t0[:], d_W1T[0:128, :])
        nc.sync.dma_start(w1t1[:], d_W1T[128 : H + 1, :])
        nc.sync.dma_start(w2t[:], d_W2T[:, :])
        nc.sync.dma_start(w3t[:], d_W3T[:, :])
        nc.sync.dma_start(c0_sb[:], d_c0[:, :])

        for bb in range(10, NB):
            en = encN_pool.tile(
                [128, NCH * H], dt.bfloat16, name=f"en{bb}", tag="en"
            )
            nc.gpsimd.dma_start(
                en[:], d_encN[bb].rearrange("(p n) h -> p (n h)", p=128)
            )
            en_tiles.append(en)

        # ---------- phase 2: softmax pieces ----------
        # p = exp(scores) in one shot (no max-subtraction: |scores| <= ~8).
        # Z_b via two tiny PE reductions; normalization deferred to the ctx
        # scale (context is linear in p).
        ct0 = spool.tile([H0, NB], dt.bfloat16)
        ct1 = spool.tile([H1 + 1, NB], dt.bfloat16)  # row 72 = ones (bias row)
        nc.scalar.dma_start(ct1[H1 : H1 + 1, :], d_ones_b[:, :])
        g0_bf = spool.tile([NB, G4], dt.bfloat16)

        with tc.tile_pool(name="z_psum", bufs=1, space="PSUM") as z_ps:
            nc.scalar.activation(p_sb[:], sc[:], AF.Exp)
            # per-(batch,chunk) partition sums: zc[ch, b]
            zc = z_ps.tile([NCH, NB], dt.float32, tag="zc")
            for b in range(NB):
                nc.tensor.matmul(
                    zc[:, b : b + 1],
                    p_sb[:, b * NCH : (b + 1) * NCH],
                    ones_c_bf[:],
                    start=True,
                    stop=True,
                )
            zc_sb = spool.tile([NCH, NB], dt.float32)
            nc.vector.tensor_copy(zc_sb[:], zc[:])
            # Z per batch as a row [1, NB], then 1/Z, then broadcast to
            # [128, NB] via a K=1 outer product
            zrow = z_ps.tile([1, NB], dt.float32, tag="zrow")
            nc.tensor.matmul(
                zrow[:], ones_c_f[0:NCH, :], zc_sb[:], start=True, stop=True
            )
            rz_sb = spool.tile([1, NB], dt.float32)
            nc.vector.reciprocal(rz_sb[:], zrow[:])
            rzb = z_ps.tile([128, NB], dt.float32, tag="rzb")
            nc.tensor.matmul(rzb[:], ones_r_f[:], rz_sb[:], start=True, stop=True)
            rzb_sb = spool.tile([128, NB], dt.float32)
            nc.vector.tensor_copy(rzb_sb[:], rzb[:])

            # ---------- phase 3: context (flipped: encN stationary, N=1) ----
            ct0_ps = z_ps.tile([H0, NB], dt.float32, tag="ct0")
            ct1_ps = z_ps.tile([H1, NB], dt.float32, tag="ct1")
            for b in range(NB):
                for c in range(NCH):
                    pcol = p_sb[:, b * NCH + c : b * NCH + c + 1]
                    nc.tensor.matmul(
                        ct0_ps[:, b : b + 1],
                        en_tiles[b][:, c * H : c * H + H0],
                        pcol,
                        start=(c == 0),
                        stop=(c == NCH - 1),
                    )
                    nc.tensor.matmul(
                        ct1_ps[:, b : b + 1],
                        en_tiles[b][:, c * H + H0 : (c + 1) * H],
                        pcol,
                        start=(c == 0),
                        stop=(c == NCH - 1),
                    )
            # normalize: ctxT = ctx_raw * (1/Z) broadcast, cast bf16
            nc.vector.tensor_tensor(ct0[:], ct0_ps[:], rzb_sb[:], op=OP.mult)
            nc.vector.tensor_tensor(
                ct1[0:H1, :], ct1_ps[:], rzb_sb[0:H1, :], op=OP.mult
            )

            # ---------- phase 4: G0 ----------
            gp = z_ps.tile([NB, G4], dt.float32, tag="g0")
            for n, nsz in [(0, 512), (512, G4 - 512)]:
                nc.tensor.matmul(
                    gp[:, n : n + nsz], ct0[:], wihc0[:, n : n + nsz],
                    start=True, stop=False,
                )
                nc.tensor.matmul(
                    gp[:, n : n + nsz], ct1[:], wihc1[:, n : n + nsz],
                    start=False, stop=False,
                )
                nc.tensor.matmul(
                    gp[:, n : n + nsz], qt0[:], whh0[:, n : n + nsz],
                    start=False, stop=False,
                )
                nc.tensor.matmul(
                    gp[:, n : n + nsz], qt1[:], whh1[:, n : n + nsz],
                    start=False, stop=True,
                )
            nc.vector.tensor_copy(g0_bf[:], gp[:])

        # ---------- phase 5: decoder steps (all bf16, gate order f,i,o|g) ---
        x_sb = spool.tile([NB, 1], dt.float32)
        nc.sync.dma_start(x_sb[:], d_x0[:, :])
        ht0 = spool.tile([H0, NB], dt.bfloat16)
        ht1 = spool.tile([H1 + 1, NB], dt.bfloat16)  # row 72 = ones (b1 row)
        nc.scalar.dma_start(ht1[H1 : H1 + 1, :], d_ones_b[:, :])
        o1t = spool.tile([101, NB], dt.bfloat16)  # row 100 = ones (b2 row)
        nc.scalar.dma_start(o1t[100:101, :], d_ones_b[:, :])
        o2t = spool.tile([51, NB], dt.bfloat16)  # row 50 = ones (b3 row)
        nc.scalar.dma_start(o2t[50:51, :], d_ones_b[:, :])

        with (
            tc.tile_pool(name="ls", bufs=2) as ls,
            tc.tile_pool(name="ls_psum", bufs=3, space="PSUM") as lp,
        ):
            xt = x_sb
            for t in range(NSTEPS):
                # gates = g0 + x * wxr, fused on DVE
                gates = ls.tile([NB, G4], dt.bfloat16, tag="gates")
                nc.vector.scalar_tensor_tensor(
                    gates[:], wxr_sb[:], xt[:, 0:1], g0_bf[:],
                    op0=OP.mult, op1=OP.add,
                )
                # f,i,o sigmoid in one instr; g tanh
                sfio = ls.tile([NB, 3 * H], dt.bfloat16, tag="sfio")
                nc.scalar.activation(sfio[:], gates[:, 0 : 3 * H], AF.Sigmoid)
                tg = ls.tile([NB, H], dt.bfloat16, tag="tg")
                nc.scalar.activation(tg[:], gates[:, 3 * H : 4 * H], AF.Tanh)
                t1 = ls.tile([NB, H], dt.bfloat16, tag="t1")
                nc.vector.tensor_tensor(t1[:], sfio[:, 0:H], c0_sb[:], op=OP.mult)
                t2 = ls.tile([NB, H], dt.bfloat16, tag="t2")
                nc.vector.tensor_tensor(t2[:], sfio[:, H : 2 * H], tg[:], op=OP.mult)
                cn = ls.tile([NB, H], dt.bfloat16, tag="cn")
                nc.vector.tensor_tensor(cn[:], t1[:], t2[:], op=OP.add)
                tcn = ls.tile([NB, H], dt.bfloat16, tag="tcn")
                nc.scalar.activation(tcn[:], cn[:], AF.Tanh)
                hh = ls.tile([NB, H], dt.bfloat16, tag="hh")
                nc.vector.tensor_tensor(hh[:], sfio[:, 2 * H : 3 * H], tcn[:], op=OP.mult)
                # transpose h -> ht0/ht1 (feature-major for the MLP); relu
                # folded into the PSUM->SBUF max-copies on DVE
                tp0 = lp.tile([128, NB], dt.bfloat16, tag="lsps")
                nc.tensor.transpose(tp0[:], hh[:, 0:H0], id_bf[0:NB, 0:NB])
                nc.vector.tensor_scalar_max(ht0[:], tp0[:], 0.0)
                tp1 = lp.tile([128, NB], dt.bfloat16, tag="lsps")
                nc.tensor.transpose(tp1[0:H1, :], hh[:, H0:H], id_bf[0:NB, 0:NB])
                nc.vector.tensor_scalar_max(ht1[0:H1, :], tp1[0:H1, :], 0.0)
                # MLP in feature-major, biases via ones rows
                m1 = lp.tile([100, NB], dt.float32, tag="lsps")
                nc.tensor.matmul(m1[:], w1t0[:], ht0[:], start=True, stop=False)
                nc.tensor.matmul(m1[:], w1t1[:], ht1[:], start=False, stop=True)
                nc.vector.tensor_scalar_max(o1t[0:100, :], m1[:], 0.0)
                m2 = lp.tile([50, NB], dt.float32, tag="lsps")
                nc.tensor.matmul(m2[:], w2t[:], o1t[:], start=True, stop=True)
                nc.vector.tensor_scalar_max(o2t[0:50, :], m2[:], 0.0)
                # m3 flipped: o2 stationary, w3 moving -> out [NB, 1] is
                # directly the next step's x (and this step's y)
                m3 = lp.tile([NB, 1], dt.float32, tag="lsps")
                nc.tensor.matmul(m3[:], o2t[:], w3t[:], start=True, stop=True)
                xn = ls.tile([NB, 1], dt.float32, tag="xn")
                nc.vector.tensor_copy(xn[:], m3[:])
                nc.sync.dma_start(d_y[:, t : t + 1], xn[:])
                xt = xn

    # Bacc lowering: register allocation + wait splitting (<=1 wait/inst on HW)
    nc.compile()
    return nc


def _prep_inputs(x, h0, c0, encoder_output, Wa, ba, Ua, bua, Va, bva,
                 W_ih, W_hh, b_ih, b_hh, W1, b1, W2, b2, W3, b3):
    """Host-side layout prep -> list of per-core input maps."""
    f32 = np.float32
    enc = np.ascontiguousarray(encoder_output, dtype=f32)
    q = np.asarray(h0, dtype=f32)[0]          # [B, H]
    c0f = np.asarray(c0, dtype=f32)[0]        # [B, H]
    x0 = np.asarray(x, dtype=f32).reshape(B, 1)

    # gate permutation: torch order (i,f,g,o) -> device order (f,i,o,g) so
    # one sigmoid instr covers f,i,o and tanh covers g
    gperm = np.concatenate(
        [np.arange(H, 2 * H), np.arange(0, H), np.arange(3 * H, 4 * H),
         np.arange(2 * H, 3 * H)]
    )
    W_ihp = np.asarray(W_ih, f32)[gperm]
    W_hhp = np.asarray(W_hh, f32)[gperm]
    bp = (np.asarray(b_ih, f32) + np.asarray(b_hh, f32))[gperm]

    # replicated weights (shared by every core)
    shared = {
        "UaT": np.ascontiguousarray(np.asarray(Ua, f32).T).astype(BF16),
        "WaT": np.ascontiguousarray(np.asarray(Wa, f32).T).astype(BF16),
        "qb": (np.asarray(ba, f32) + np.asarray(bua, f32)).reshape(H, 1),
        "VaT": np.ascontiguousarray(np.asarray(Va, f32)[0].reshape(H, 1)).astype(BF16),
        "WihcT": np.concatenate(
            [W_ihp[:, 1:].T, bp.reshape(1, G4)], axis=0
        ).astype(BF16),
        "WhhT": np.ascontiguousarray(W_hhp.T).astype(BF16),
        "wxr": np.broadcast_to(W_ihp[:, 0].reshape(1, G4), (NB, G4)).astype(BF16),
        "W1T": np.concatenate(
            [np.asarray(W1, f32).T, np.asarray(b1, f32).reshape(1, 100)], axis=0
        ).astype(BF16),
        "W2T": np.concatenate(
            [np.asarray(W2, f32).T, np.asarray(b2, f32).reshape(1, 50)], axis=0
        ).astype(BF16),
        "W3T": np.concatenate(
            [np.asarray(W3, f32).T, np.asarray(b3, f32).reshape(1, 1)], axis=0
        ).astype(BF16),
        "ones_b": np.ones((1, NB), BF16),
    }

    in_maps = []
    for c in range(NCORES):
        bs = slice(c * NB, (c + 1) * NB)
        enc_c = enc[bs]  # [NB, T, H]
        m = dict(shared)
        m["encT"] = np.ascontiguousarray(enc_c.transpose(0, 2, 1)).astype(BF16)
        m["encN"] = enc_c.astype(BF16)
        m["qT"] = np.ascontiguousarray(q[bs].T).astype(BF16)
        m["c0s"] = np.ascontiguousarray(c0f[bs]).astype(BF16)
        m["x0s"] = np.ascontiguousarray(x0[bs])
        in_maps.append(m)
    return in_maps


def kernel(**inputs):
    from concourse.bass_utils import run_bass_kernel_spmd

    if "nc" not in _CACHE:
        _CACHE["nc"] = _build_module()
    nc = _CACHE["nc"]

    in_maps = _prep_inputs(**inputs)
    res = run_bass_kernel_spmd(nc, in_maps, core_ids=list(range(NCORES)))
    # y2 per core: [NB, NSTEPS] -> full output [B, NSTEPS]
    out = np.concatenate([r["y2"] for r in res.results], axis=0)
    return np.ascontiguousarray(out.astype(np.float32))


# revision 6
# speedup vs baseline: 1.5132x; 1.1387x over previous
"""Trainium2 Bass kernel for nn_DecoderAttention (Bahdanau attention + LSTM decoder).

Data-parallel over batch: B=128 split across 8 NeuronCores (16 batches/core).
All FLOPs run on device; the host only reshuffles layouts (transpose / dtype
cast / weight concat with bias rows folded in as an extra contraction row).

Per-core device pipeline (cost-model-aware layout):
  phase 0: load weights, identity, qprojT = Wa @ q^T (+ ba + bua) on PE
  phase 1: per batch b: kprojT = Ua @ enc_b^T on PE (bf16 in, fp32 PSUM),
           tanh(kprojT + qprojT[:, b]) on ACT -> e tiles [h, t] in SBUF;
           scores via FLIPPED matmuls: e chunk stationary (K=h, M=128 t's of
           one stride class), Va moving (N=1) -> scores land [t, (b,chunk)]
           columns of ONE PSUM tile; N=1 makes these matmuls ~free and kills
           the pT transpose phase entirely.
  phase 2: ONE exp over the [128, 256] scores tile -> p (bf16, unnormalized);
           Z per batch via two tiny PE reductions (colsum then chunk-sum),
           broadcast 1/Z to [128, 16] via a K=1 outer-product matmul.
  phase 3: context via FLIPPED matmuls: encN chunk stationary (K=t, M=h),
           p column moving (N=1) -> ctxT [h, b] accumulates directly in the
           layout G0 needs (no transposes); scale by 1/Z on DVE.
  phase 4: G0 = ctx @ W_ihc^T + q @ W_hh^T + (b_ih + b_hh) on PE
  phase 5: 5 serial decoder steps, all-bf16 elementwise:
           gates = G0 + x*wxr fused on DVE (scalar_tensor_tensor),
           gate order permuted to (f,i,o | g) so one sigmoid covers f,i,o,
           MLP in bf16 with m3 operands swapped so x_next = out [16, 1].
"""

import numpy as np
import ml_dtypes

B, T, H = 128, 2048, 200
NCORES = 8
NB = B // NCORES  # 16 batches per core
NSTEPS = 5
G4 = 4 * H  # 800 gate width
NCH = T // 128  # 16 stride-class chunks (t = 16*k + c -> partition k, chunk c)

_CACHE = {}

BF16 = ml_dtypes.bfloat16


def _build_module():
    """Build the Bass module (same NEFF for all 8 cores)."""
    from contextlib import ExitStack

    import concourse.bass as bass
    import concourse.tile as tile
    from concourse import bacc, mybir
    from concourse.masks import make_identity

    dt = mybir.dt
    AF = mybir.ActivationFunctionType
    OP = mybir.AluOpType

    nc = bacc.Bacc(
        "TRN2",
        target_bir_lowering=False,
        debug=False,
        num_devices=NCORES,
    )

    # ---- DRAM tensors (per-core shards; weights replicated) ----
    d_encT = nc.dram_tensor("encT", [NB, H, T], dt.bfloat16, kind="ExternalInput").ap()
    d_encN = nc.dram_tensor("encN", [NB, T, H], dt.bfloat16, kind="ExternalInput").ap()
    d_qT = nc.dram_tensor("qT", [H, NB], dt.bfloat16, kind="ExternalInput").ap()
    d_c0 = nc.dram_tensor("c0s", [NB, H], dt.bfloat16, kind="ExternalInput").ap()
    d_x0 = nc.dram_tensor("x0s", [NB, 1], dt.float32, kind="ExternalInput").ap()
    d_UaT = nc.dram_tensor("UaT", [H, H], dt.bfloat16, kind="ExternalInput").ap()
    d_WaT = nc.dram_tensor("WaT", [H, H], dt.bfloat16, kind="ExternalInput").ap()
    d_qb = nc.dram_tensor("qb", [H, 1], dt.float32, kind="ExternalInput").ap()
    d_VaT = nc.dram_tensor("VaT", [H, 1], dt.bfloat16, kind="ExternalInput").ap()
    d_WihcT = nc.dram_tensor(
        "WihcT", [H + 1, G4], dt.bfloat16, kind="ExternalInput"
    ).ap()
    d_WhhT = nc.dram_tensor("WhhT", [H, G4], dt.bfloat16, kind="ExternalInput").ap()
    d_wxr = nc.dram_tensor("wxr", [NB, G4], dt.bfloat16, kind="ExternalInput").ap()
    d_W1T = nc.dram_tensor("W1T", [H + 1, 100], dt.bfloat16, kind="ExternalInput").ap()
    d_W2T = nc.dram_tensor("W2T", [101, 50], dt.bfloat16, kind="ExternalInput").ap()
    d_W3T = nc.dram_tensor("W3T", [51, 1], dt.bfloat16, kind="ExternalInput").ap()
    # ones rows for the bias-row (aug) trick; DMA'd because compute engines
    # cannot write at non-32-aligned partition offsets
    d_ones_b = nc.dram_tensor("ones_b", [1, NB], dt.bfloat16, kind="ExternalInput").ap()
    d_y = nc.dram_tensor("y2", [NB, NSTEPS], dt.float32, kind="ExternalOutput").ap()

    H0, H1 = 128, H - 128  # 128 + 72 partition chunks of the hidden dim

    with tile.TileContext(nc) as tc, ExitStack() as ctx:
        # ---------- persistent pools ----------
        wpool = ctx.enter_context(tc.tile_pool(name="weights", bufs=1))
        spool = ctx.enter_context(tc.tile_pool(name="smalls", bufs=1))

        # warmup: preload the ACT table sets (tanh/exp + sigmoid) while DMAs
        # stream, so no table load lands mid-kernel
        wt_a = spool.tile([1, 8], dt.float32)
        nc.vector.memset(wt_a[:], 0.0)
        wt_b = spool.tile([1, 8], dt.float32)
        nc.scalar.activation(wt_b[:], wt_a[:], AF.Tanh)
        nc.scalar.activation(wt_b[:], wt_a[:], AF.Sigmoid)
        nc.scalar.activation(wt_b[:], wt_a[:], AF.Exp)

        # identity for the decoder's h transposes (bf16)
        id_bf = wpool.tile([128, 128], dt.bfloat16)
        make_identity(nc, id_bf[:])

        # ones columns/rows for the tiny PE reductions (sliced on read; memset
        # writes full 128-partition tiles so partition offsets stay aligned)
        ones_c_bf = wpool.tile([128, 1], dt.bfloat16)
        nc.vector.memset(ones_c_bf[:], 1.0)
        ones_c_f = wpool.tile([128, 1], dt.float32)
        nc.vector.memset(ones_c_f[:], 1.0)
        ones_r_f = wpool.tile([1, 128], dt.float32)
        nc.vector.memset(ones_r_f[:], 1.0)

        # attention weights
        ua0 = wpool.tile([H0, H], dt.bfloat16)
        ua1 = wpool.tile([H1, H], dt.bfloat16)
        wa0 = wpool.tile([H0, H], dt.bfloat16)
        wa1 = wpool.tile([H1, H], dt.bfloat16)
        nc.scalar.dma_start(wa0[:], d_WaT[0:H0, :])
        nc.scalar.dma_start(wa1[:], d_WaT[H0:H, :])
        va0 = wpool.tile([H0, 1], dt.bfloat16)
        va1 = wpool.tile([H1, 1], dt.bfloat16)
        nc.scalar.dma_start(va0[:], d_VaT[0:H0, :])
        nc.scalar.dma_start(va1[:], d_VaT[H0:H, :])
        qt0 = wpool.tile([H0, NB], dt.bfloat16)
        qt1 = wpool.tile([H1, NB], dt.bfloat16)
        nc.scalar.dma_start(qt0[:], d_qT[0:H0, :])
        nc.scalar.dma_start(qt1[:], d_qT[H0:H, :])
        qb0 = wpool.tile([H0, 1], dt.float32)
        qb1 = wpool.tile([H1, 1], dt.float32)
        nc.scalar.dma_start(qb0[:], d_qb[0:H0, :])
        nc.scalar.dma_start(qb1[:], d_qb[H0:H, :])

        # decoder weights (allocated now, DMA'd later to keep the SP DGE ring
        # clear for encT during the attention phase)
        wihc0 = wpool.tile([128, G4], dt.bfloat16)
        wihc1 = wpool.tile([73, G4], dt.bfloat16)
        whh0 = wpool.tile([H0, G4], dt.bfloat16)
        whh1 = wpool.tile([H1, G4], dt.bfloat16)
        wxr_sb = wpool.tile([NB, G4], dt.bfloat16)
        w1t0 = wpool.tile([128, 100], dt.bfloat16)
        w1t1 = wpool.tile([73, 100], dt.bfloat16)
        w2t = wpool.tile([101, 50], dt.bfloat16)
        w3t = wpool.tile([51, 1], dt.bfloat16)
        c0_sb = spool.tile([NB, H], dt.bfloat16)

        # ---------- phase 0: qprojT = Wa @ q^T + (ba + bua) ----------
        qproj0 = spool.tile([H0, NB], dt.float32)  # fp32 bias tiles for tanh
        qproj1 = spool.tile([H1, NB], dt.float32)
        with tc.tile_pool(name="qp_psum", bufs=1, space="PSUM") as qp_ps:
            for m, (msz, qdst, qb) in enumerate(
                [(H0, qproj0, qb0), (H1, qproj1, qb1)]
            ):
                mlo = m * H0
                ps = qp_ps.tile([128, NB], dt.float32, tag="qp")
                nc.tensor.matmul(
                    ps[0:msz, :], wa0[:, mlo : mlo + msz], qt0[:], start=True, stop=False
                )
                nc.tensor.matmul(
                    ps[0:msz, :], wa1[:, mlo : mlo + msz], qt1[:], start=False, stop=True
                )
                nc.vector.tensor_scalar_add(qdst[:], ps[0:msz, :], qb[:])

        # ---------- phase 1: kproj + tanh + scores (flipped) ----------
        # scores land in ONE [128, NB*NCH] PSUM tile: column 16*b + c holds
        # the 128 t's of stride class c (t = 16*k + c -> partition k), which
        # matches the encN "(p n) h" SBUF layout used by the context matmuls.
        encT_pool = ctx.enter_context(tc.tile_pool(name="encT_pool", bufs=3))
        e_pool = ctx.enter_context(tc.tile_pool(name="e_pool", bufs=4))
        encN_pool = ctx.enter_context(tc.tile_pool(name="encN_pool", bufs=16))
        sc_pool = ctx.enter_context(tc.tile_pool(name="sc_psum", bufs=1, space="PSUM"))
        sc = sc_pool.tile([128, NB * NCH], dt.float32, tag="sc")
        p_sb = spool.tile([128, NB * NCH], dt.bfloat16)

        en_tiles = []
        e_saved = []  # (e0, e1) per batch, scores emitted with 1-batch skew

        def emit_scores(b, e0, e1):
            for c in range(NCH):
                col = NB * 0 + b * NCH + c
                nc.tensor.matmul(
                    sc[:, col : col + 1],
                    e0[:, c : T : NCH],
                    va0[:],
                    start=True,
                    stop=False,
                )
                nc.tensor.matmul(
                    sc[:, col : col + 1],
                    e1[:, c : T : NCH],
                    va1[:],
                    start=False,
                    stop=True,
                )

        nc.sync.dma_start(ua0[:], d_UaT[0:H0, :])
        nc.sync.dma_start(ua1[:], d_UaT[H0:H, :])
        with tc.tile_pool(name="kp_psum", bufs=3, space="PSUM") as kp_ps:
            for b in range(NB):
                et0 = encT_pool.tile([H0, T], dt.bfloat16, tag="et0")
                et1 = encT_pool.tile([H1, T], dt.bfloat16, tag="et1")
                nc.sync.dma_start(et0[:], d_encT[b, 0:H0, :])
                i_et1 = nc.sync.dma_start(et1[:], d_encT[b, H0:H, :])
                e0 = e_pool.tile([H0, T], dt.bfloat16, tag="e0")
                e1 = e_pool.tile([H1, T], dt.bfloat16, tag="e1")
                i_kp = None
                for m, (msz, edst, qp) in enumerate(
                    [(H0, e0, qproj0), (H1, e1, qproj1)]
                ):
                    mlo = m * H0
                    for th in range(2):  # 1024-wide psum tiles
                        ps = kp_ps.tile([128, 1024], dt.float32, tag="kp")
                        for n in range(2):
                            c0c = th * 1024 + n * 512
                            nc.tensor.matmul(
                                ps[0:msz, n * 512 : (n + 1) * 512],
                                ua0[:, mlo : mlo + msz],
                                et0[:, c0c : c0c + 512],
                                start=True,
                                stop=False,
                            )
                            i_kp = nc.tensor.matmul(
                                ps[0:msz, n * 512 : (n + 1) * 512],
                                ua1[:, mlo : mlo + msz],
                                et1[:, c0c : c0c + 512],
                                start=False,
                                stop=True,
                            )
                        # e = tanh(kproj + qproj[:, b]) ; write bf16
                        nc.scalar.activation(
                            edst[:, th * 1024 : (th + 1) * 1024],
                            ps[0:msz, :],
                            AF.Tanh,
                            bias=qp[:, b : b + 1],
                        )
                # scores with a 1-batch skew so the PE never stalls waiting on
                # this batch's tanh while the next batch's kproj is runnable
                if b > 0:
                    emit_scores(b - 1, *e_saved[b - 1])
                e_saved.append((e0, e1))
                # encN loads paced on the (otherwise idle) SWDGE ring,
                # gated behind this batch's kproj to keep encT priority
                import bass_rust as _br

                en = encN_pool.tile(
                    [128, NCH * H], dt.bfloat16, name=f"en{b}", tag="en"
                )
                i_en = nc.gpsimd.dma_start(
                    en[:],
                    d_encN[b].rearrange("(p n) h -> p (n h)", p=128),
                )
                _br.add_dep_helper(
                    i_en.ins, i_kp.ins, sync=True,
                    reason="encN paced behind this batch's kproj",
                )
                en_tiles.append(en)
            emit_scores(NB - 1, *e_saved[NB - 1])

        # deferred decoder-weight loads (SP ring is now free)
        nc.sync.dma_start(wihc0[:], d_WihcT[0:128, :])
        nc.sync.dma_start(wihc1[:], d_WihcT[128 : H + 1, :])
        nc.sync.dma_start(whh0[:], d_WhhT[0:H0, :])
        nc.sync.dma_start(whh1[:], d_WhhT[H0:H, :])
        nc.sync.dma_start(wxr_sb[:], d_wxr[:, :])
        nc.sync.dma_start(w1t0[:], d_W1T[0:128, :])
        nc.sync.dma_start(w1t1[:], d_W1T[128 : H + 1, :])
        nc.sync.dma_start(w2t[:], d_W2T[:, :])
        nc.sync.dma_start(w3t[:], d_W3T[:, :])
        nc.sync.dma_start(c0_sb[:], d_c0[:, :])

        # ---------- phase 2: softmax pieces ----------
        # p = exp(scores) in one shot (no max-subtraction: |scores| <= ~8).
        # Z_b via two tiny PE reductions; normalization deferred to the ctx
        # scale (context is linear in p).
        ct0 = spool.tile([H0, NB], dt.bfloat16)
        ct1 = spool.tile([H1 + 1, NB], dt.bfloat16)  # row 72 = ones (bias row)
        nc.scalar.dma_start(ct1[H1 : H1 + 1, :], d_ones_b[:, :])
        g0_pool = ctx.enter_context(tc.tile_pool(name="g0_psum", bufs=1, space="PSUM"))
        gp = g0_pool.tile([NB, G4], dt.float32, tag="g0")

        with tc.tile_pool(name="z_psum", bufs=1, space="PSUM") as z_ps:
            nc.scalar.activation(p_sb[:], sc[:], AF.Exp)
            # per-(batch,chunk) partition sums: zc[ch, b]
            zc = z_ps.tile([NCH, NB], dt.float32, tag="zc")
            for b in range(NB):
                nc.tensor.matmul(
                    zc[:, b : b + 1],
                    p_sb[:, b * NCH : (b + 1) * NCH],
                    ones_c_bf[:],
                    start=True,
                    stop=True,
                )
            zc_sb = spool.tile([NCH, NB], dt.float32)
            nc.vector.tensor_copy(zc_sb[:], zc[:])
            # Z per batch as a row [1, NB], then 1/Z, then broadcast to
            # [128, NB] via a K=1 outer product
            zrow = z_ps.tile([1, NB], dt.float32, tag="zrow")
            nc.tensor.matmul(
                zrow[:], ones_c_f[0:NCH, :], zc_sb[:], start=True, stop=True
            )
            rz_sb = spool.tile([1, NB], dt.float32)
            nc.vector.reciprocal(rz_sb[:], zrow[:])
            rzb = z_ps.tile([128, NB], dt.float32, tag="rzb")
            nc.tensor.matmul(rzb[:], ones_r_f[:], rz_sb[:], start=True, stop=True)
            rzb_sb = spool.tile([128, NB], dt.float32)
            nc.vector.tensor_copy(rzb_sb[:], rzb[:])

            # ---------- phase 3: context (flipped: encN stationary, N=1) ----
            ct0_ps = z_ps.tile([H0, NB], dt.float32, tag="ct0")
            ct1_ps = z_ps.tile([H1, NB], dt.float32, tag="ct1")
            for b in range(NB):
                for c in range(NCH):
                    pcol = p_sb[:, b * NCH + c : b * NCH + c + 1]
                    nc.tensor.matmul(
                        ct0_ps[:, b : b + 1],
                        en_tiles[b][:, c * H : c * H + H0],
                        pcol,
                        start=(c == 0),
                        stop=(c == NCH - 1),
                    )
                    nc.tensor.matmul(
                        ct1_ps[:, b : b + 1],
                        en_tiles[b][:, c * H + H0 : (c + 1) * H],
                        pcol,
                        start=(c == 0),
                        stop=(c == NCH - 1),
                    )
            # normalize: ctxT = ctx_raw * (1/Z) broadcast, cast bf16
            nc.vector.tensor_tensor(ct0[:], ct0_ps[:], rzb_sb[:], op=OP.mult)
            nc.vector.tensor_tensor(
                ct1[0:H1, :], ct1_ps[:], rzb_sb[0:H1, :], op=OP.mult
            )

            # ---------- phase 4: G0 (stays in PSUM for the decoder) ----
            for n, nsz in [(0, 512), (512, G4 - 512)]:
                nc.tensor.matmul(
                    gp[:, n : n + nsz], ct0[:], wihc0[:, n : n + nsz],
                    start=True, stop=False,
                )
                nc.tensor.matmul(
                    gp[:, n : n + nsz], ct1[:], wihc1[:, n : n + nsz],
                    start=False, stop=False,
                )
                nc.tensor.matmul(
                    gp[:, n : n + nsz], qt0[:], whh0[:, n : n + nsz],
                    start=False, stop=False,
                )
                nc.tensor.matmul(
                    gp[:, n : n + nsz], qt1[:], whh1[:, n : n + nsz],
                    start=False, stop=True,
                )

        # ---------- phase 5: decoder steps (all bf16, gate order f,i,o|g) ---
        x_sb = spool.tile([NB, 1], dt.float32)
        nc.sync.dma_start(x_sb[:], d_x0[:, :])
        xn_all = spool.tile([NB, NSTEPS], dt.float32)
        ht0 = spool.tile([H0, NB], dt.bfloat16)
        ht1 = spool.tile([H1 + 1, NB], dt.bfloat16)  # row 72 = ones (b1 row)
        nc.scalar.dma_start(ht1[H1 : H1 + 1, :], d_ones_b[:, :])
        o1t = spool.tile([101, NB], dt.bfloat16)  # row 100 = ones (b2 row)
        nc.scalar.dma_start(o1t[100:101, :], d_ones_b[:, :])
        o2t = spool.tile([51, NB], dt.bfloat16)  # row 50 = ones (b3 row)
        nc.scalar.dma_start(o2t[50:51, :], d_ones_b[:, :])

        with (
            tc.tile_pool(name="ls", bufs=2) as ls,
            tc.tile_pool(name="ls_psum", bufs=3, space="PSUM") as lp,
        ):
            xt = x_sb
            for t in range(NSTEPS):
                # gates = g0 + x * wxr, fused on DVE; split f,i,o vs g so
                # the sigmoid can start before the g slice is computed
                gates = ls.tile([NB, G4], dt.bfloat16, tag="gates")
                nc.vector.scalar_tensor_tensor(
                    gates[:, 0 : 3 * H], wxr_sb[:, 0 : 3 * H], xt[:, 0:1],
                    gp[:, 0 : 3 * H], op0=OP.mult, op1=OP.add,
                )
                nc.vector.scalar_tensor_tensor(
                    gates[:, 3 * H : 4 * H], wxr_sb[:, 3 * H : 4 * H], xt[:, 0:1],
                    gp[:, 3 * H : 4 * H], op0=OP.mult, op1=OP.add,
                )
                # f,i,o sigmoid in one instr; g tanh
                sfio = ls.tile([NB, 3 * H], dt.bfloat16, tag="sfio")
                nc.scalar.activation(sfio[:], gates[:, 0 : 3 * H], AF.Sigmoid)
                tg = ls.tile([NB, H], dt.bfloat16, tag="tg")
                nc.scalar.activation(tg[:], gates[:, 3 * H : 4 * H], AF.Tanh)
                t1 = ls.tile([NB, H], dt.bfloat16, tag="t1")
                nc.vector.tensor_tensor(t1[:], sfio[:, 0:H], c0_sb[:], op=OP.mult)
                t2 = ls.tile([NB, H], dt.bfloat16, tag="t2")
                nc.vector.tensor_tensor(t2[:], sfio[:, H : 2 * H], tg[:], op=OP.mult)
                cn = ls.tile([NB, H], dt.bfloat16, tag="cn")
                nc.vector.tensor_tensor(cn[:], t1[:], t2[:], op=OP.add)
                tcn = ls.tile([NB, H], dt.bfloat16, tag="tcn")
                nc.scalar.activation(tcn[:], cn[:], AF.Tanh)
                hh = ls.tile([NB, H], dt.bfloat16, tag="hh")
                nc.vector.tensor_tensor(hh[:], sfio[:, 2 * H : 3 * H], tcn[:], op=OP.mult)
                # transpose h -> ht0/ht1 (feature-major for the MLP); relu
                # folded into the PSUM->SBUF max-copies on DVE
                tp0 = lp.tile([128, NB], dt.bfloat16, tag="lsps")
                nc.tensor.transpose(tp0[:], hh[:, 0:H0], id_bf[0:NB, 0:NB])
                nc.vector.tensor_scalar_max(ht0[:], tp0[:], 0.0)
                tp1 = lp.tile([128, NB], dt.bfloat16, tag="lsps")
                nc.tensor.transpose(tp1[0:H1, :], hh[:, H0:H], id_bf[0:NB, 0:NB])
                nc.scalar.activation(ht1[0:H1, :], tp1[0:H1, :], AF.Relu)
                # MLP in feature-major, biases via ones rows
                m1 = lp.tile([100, NB], dt.float32, tag="lsps")
                nc.tensor.matmul(m1[:], w1t0[:], ht0[:], start=True, stop=False)
                nc.tensor.matmul(m1[:], w1t1[:], ht1[:], start=False, stop=True)
                nc.vector.tensor_scalar_max(o1t[0:100, :], m1[:], 0.0)
                m2 = lp.tile([50, NB], dt.float32, tag="lsps")
                nc.tensor.matmul(m2[:], w2t[:], o1t[:], start=True, stop=True)
                nc.vector.tensor_scalar_max(o2t[0:50, :], m2[:], 0.0)
                # m3 flipped: o2 stationary, w3 moving -> out [NB, 1] is
                # directly the next step's x (and this step's y)
                m3 = lp.tile([NB, 1], dt.float32, tag="lsps")
                nc.tensor.matmul(m3[:], o2t[:], w3t[:], start=True, stop=True)
                nc.vector.tensor_copy(xn_all[:, t : t + 1], m3[:])
                xt = xn_all[:, t : t + 1]
            nc.sync.dma_start(d_y[:, :], xn_all[:])

    # Bacc lowering: register allocation + wait splitting (<=1 wait/inst on HW)
    nc.compile()
    return nc


def _prep_inputs(x, h0, c0, encoder_output, Wa, ba, Ua, bua, Va, bva,
                 W_ih, W_hh, b_ih, b_hh, W1, b1, W2, b2, W3, b3):
    """Host-side layout prep -> list of per-core input maps."""
    f32 = np.float32
    enc = np.ascontiguousarray(encoder_output, dtype=f32)
    q = np.asarray(h0, dtype=f32)[0]          # [B, H]
    c0f = np.asarray(c0, dtype=f32)[0]        # [B, H]
    x0 = np.asarray(x, dtype=f32).reshape(B, 1)

    # gate permutation: torch order (i,f,g,o) -> device order (f,i,o,g) so
    # one sigmoid instr covers f,i,o and tanh covers g
    gperm = np.concatenate(
        [np.arange(H, 2 * H), np.arange(0, H), np.arange(3 * H, 4 * H),
         np.arange(2 * H, 3 * H)]
    )
    W_ihp = np.asarray(W_ih, f32)[gperm]
    W_hhp = np.asarray(W_hh, f32)[gperm]
    bp = (np.asarray(b_ih, f32) + np.asarray(b_hh, f32))[gperm]

    # replicated weights (shared by every core)
    shared = {
        "UaT": np.ascontiguousarray(np.asarray(Ua, f32).T).astype(BF16),
        "WaT": np.ascontiguousarray(np.asarray(Wa, f32).T).astype(BF16),
        "qb": (np.asarray(ba, f32) + np.asarray(bua, f32)).reshape(H, 1),
        "VaT": np.ascontiguousarray(np.asarray(Va, f32)[0].reshape(H, 1)).astype(BF16),
        "WihcT": np.concatenate(
            [W_ihp[:, 1:].T, bp.reshape(1, G4)], axis=0
        ).astype(BF16),
        "WhhT": np.ascontiguousarray(W_hhp.T).astype(BF16),
        "wxr": np.broadcast_to(W_ihp[:, 0].reshape(1, G4), (NB, G4)).astype(BF16),
        "W1T": np.concatenate(
            [np.asarray(W1, f32).T, np.asarray(b1, f32).reshape(1, 100)], axis=0
        ).astype(BF16),
        "W2T": np.concatenate(
            [np.asarray(W2, f32).T, np.asarray(b2, f32).reshape(1, 50)], axis=0
        ).astype(BF16),
        "W3T": np.concatenate(
            [np.asarray(W3, f32).T, np.asarray(b3, f32).reshape(1, 1)], axis=0
        ).astype(BF16),
        "ones_b": np.ones((1, NB), BF16),
    }

    in_maps = []
    for c in range(NCORES):
        bs = slice(c * NB, (c + 1) * NB)
        enc_c = enc[bs]  # [NB, T, H]
        m = dict(shared)
        m["encT"] = np.ascontiguousarray(enc_c.transpose(0, 2, 1)).astype(BF16)
        m["encN"] = enc_c.astype(BF16)
        m["qT"] = np.ascontiguousarray(q[bs].T).astype(BF16)
        m["c0s"] = np.ascontiguousarray(c0f[bs]).astype(BF16)
        m["x0s"] = np.ascontiguousarray(x0[bs])
        in_maps.append(m)
    return in_maps


def kernel(**inputs):
    from concourse.bass_utils import run_bass_kernel_spmd

    if "nc" not in _CACHE:
        _CACHE["nc"] = _build_module()
    nc = _CACHE["nc"]

    in_maps = _prep_inputs(**inputs)
    res = run_bass_kernel_spmd(nc, in_maps, core_ids=list(range(NCORES)))
    # y2 per core: [NB, NSTEPS] -> full output [B, NSTEPS]
    out = np.concatenate([r["y2"] for r in res.results], axis=0)
    return np.ascontiguousarray(out.astype(np.float32))


# revision 12
# speedup vs baseline: 1.5512x; 1.0251x over previous
"""Trainium2 Bass kernel for nn_DecoderAttention (Bahdanau attention + LSTM decoder).

Data-parallel over batch: B=128 split across 8 NeuronCores (16 batches/core).
All FLOPs run on device; the host only reshuffles layouts (transpose / dtype
cast / weight concat with bias rows folded in as an extra contraction row).

Per-core device pipeline (cost-model-aware layout):
  phase 0: load weights, identity, qprojT = Wa @ q^T (+ ba + bua) on PE
  phase 1: per batch b: kprojT = Ua @ enc_b^T on PE (bf16 in, fp32 PSUM),
           tanh(kprojT + qprojT[:, b]) on ACT -> e tiles [h, t] in SBUF;
           scores via FLIPPED matmuls: e chunk stationary (K=h, M=128 t's of
           one stride class), Va moving (N=1) -> scores land [t, (b,chunk)]
           columns of ONE PSUM tile; N=1 makes these matmuls ~free and kills
           the pT transpose phase entirely.
  phase 2: ONE exp over the [128, 256] scores tile -> p (bf16, unnormalized);
           Z per batch via two tiny PE reductions (colsum then chunk-sum),
           broadcast 1/Z to [128, 16] via a K=1 outer-product matmul.
  phase 3: context via FLIPPED matmuls: encN chunk stationary (K=t, M=h),
           p column moving (N=1) -> ctxT [h, b] accumulates directly in the
           layout G0 needs (no transposes); scale by 1/Z on DVE.
  phase 4: G0 = ctx @ W_ihc^T + q @ W_hh^T + (b_ih + b_hh) on PE
  phase 5: 5 serial decoder steps, all-bf16 elementwise:
           gates = G0 + x*wxr fused on DVE (scalar_tensor_tensor),
           gate order permuted to (f,i,o | g) so one sigmoid covers f,i,o,
           MLP in bf16 with m3 operands swapped so x_next = out [16, 1].
"""

import numpy as np
import ml_dtypes

B, T, H = 128, 2048, 200
NCORES = 8
NB = B // NCORES  # 16 batches per core
NSTEPS = 5
G4 = 4 * H  # 800 gate width
NCH = T // 128  # 16 stride-class chunks (t = 16*k + c -> partition k, chunk c)

_CACHE = {}

BF16 = ml_dtypes.bfloat16


def _build_module():
    """Build the Bass module (same NEFF for all 8 cores)."""
    from contextlib import ExitStack

    import concourse.bass as bass
    import concourse.tile as tile
    from concourse import bacc, mybir
    from concourse.masks import make_identity

    dt = mybir.dt
    AF = mybir.ActivationFunctionType
    OP = mybir.AluOpType

    nc = bacc.Bacc(
        "TRN2",
        target_bir_lowering=False,
        debug=False,
        num_devices=NCORES,
    )

    # ---- DRAM tensors (per-core shards; weights replicated) ----
    d_encT = nc.dram_tensor(
        "encTp", [NB, 128, 2 * T], dt.float8e4, kind="ExternalInput"
    ).ap()  # DoubleRow packing: col i*T+t, partition p <-> h_in = i*128+p
    # (i=1, p>=72 slots are zero; dual-fp8 ldweights needs all 128 partitions)
    d_encN = nc.dram_tensor("encN", [NB, T, H], dt.bfloat16, kind="ExternalInput").ap()
    d_qT = nc.dram_tensor("qT", [H, NB], dt.bfloat16, kind="ExternalInput").ap()
    d_c0 = nc.dram_tensor("c0s", [NB, H], dt.bfloat16, kind="ExternalInput").ap()
    d_x0 = nc.dram_tensor("x0s", [NB, 1], dt.float32, kind="ExternalInput").ap()
    d_UaT = nc.dram_tensor("UaTp", [128, 2 * 208], dt.float8e4, kind="ExternalInput").ap()
    # M padded 200->208 so the DoubleRow pair stride is 16B-aligned
    d_WaT = nc.dram_tensor("WaT", [H, H], dt.bfloat16, kind="ExternalInput").ap()
    d_qb = nc.dram_tensor("qb", [H, 1], dt.float32, kind="ExternalInput").ap()
    d_VaT = nc.dram_tensor("VaT", [H, 1], dt.bfloat16, kind="ExternalInput").ap()
    d_WihcT = nc.dram_tensor(
        "WihcT", [H + 1, G4], dt.bfloat16, kind="ExternalInput"
    ).ap()
    d_WhhT = nc.dram_tensor("WhhT", [H, G4], dt.bfloat16, kind="ExternalInput").ap()
    d_wxr = nc.dram_tensor("wxr", [NB, G4], dt.bfloat16, kind="ExternalInput").ap()
    d_W1T = nc.dram_tensor("W1T", [H + 1, 100], dt.bfloat16, kind="ExternalInput").ap()
    d_W2T = nc.dram_tensor("W2T", [101, 50], dt.bfloat16, kind="ExternalInput").ap()
    d_W3T = nc.dram_tensor("W3T", [51, 1], dt.bfloat16, kind="ExternalInput").ap()
    # ones rows for the bias-row (aug) trick; DMA'd because compute engines
    # cannot write at non-32-aligned partition offsets
    d_ones_b = nc.dram_tensor("ones_b", [1, NB], dt.bfloat16, kind="ExternalInput").ap()
    d_y = nc.dram_tensor("y2", [NB, NSTEPS], dt.float32, kind="ExternalOutput").ap()

    H0, H1 = 128, H - 128  # 128 + 72 partition chunks of the hidden dim

    with tile.TileContext(nc) as tc, ExitStack() as ctx:
        # ---------- persistent pools ----------
        wpool = ctx.enter_context(tc.tile_pool(name="weights", bufs=1))
        spool = ctx.enter_context(tc.tile_pool(name="smalls", bufs=1))

        # warmup: preload the ACT table sets (tanh/exp + sigmoid) while DMAs
        # stream, so no table load lands mid-kernel
        wt_a = spool.tile([1, 8], dt.float32)
        nc.vector.memset(wt_a[:], 0.0)
        wt_b = spool.tile([1, 8], dt.float32)
        nc.scalar.activation(wt_b[:], wt_a[:], AF.Tanh)
        nc.scalar.activation(wt_b[:], wt_a[:], AF.Sigmoid)
        nc.scalar.activation(wt_b[:], wt_a[:], AF.Exp)

        # identity for the decoder's h transposes (bf16)
        id_bf = wpool.tile([128, 128], dt.bfloat16)
        make_identity(nc, id_bf[:])

        # ones columns/rows for the tiny PE reductions (sliced on read; memset
        # writes full 128-partition tiles so partition offsets stay aligned)
        ones_c_bf = wpool.tile([128, 1], dt.bfloat16)
        nc.vector.memset(ones_c_bf[:], 1.0)
        ones_c_f = wpool.tile([128, 1], dt.float32)
        nc.vector.memset(ones_c_f[:], 1.0)
        ones_r_f = wpool.tile([1, 128], dt.float32)
        nc.vector.memset(ones_r_f[:], 1.0)

        # attention weights (Ua fp8, x64 scaled, DoubleRow-packed)
        ua_p = wpool.tile([128, 2 * 208], dt.float8e4)
        wa0 = wpool.tile([H0, H], dt.bfloat16)
        wa1 = wpool.tile([H1, H], dt.bfloat16)
        nc.scalar.dma_start(wa0[:], d_WaT[0:H0, :])
        nc.scalar.dma_start(wa1[:], d_WaT[H0:H, :])
        va0 = wpool.tile([H0, 1], dt.bfloat16)
        va1 = wpool.tile([H1, 1], dt.bfloat16)
        nc.scalar.dma_start(va0[:], d_VaT[0:H0, :])
        nc.scalar.dma_start(va1[:], d_VaT[H0:H, :])
        qt0 = wpool.tile([H0, NB], dt.bfloat16)
        qt1 = wpool.tile([H1, NB], dt.bfloat16)
        nc.scalar.dma_start(qt0[:], d_qT[0:H0, :])
        nc.scalar.dma_start(qt1[:], d_qT[H0:H, :])
        qb0 = wpool.tile([H0, 1], dt.float32)
        qb1 = wpool.tile([H1, 1], dt.float32)
        nc.scalar.dma_start(qb0[:], d_qb[0:H0, :])
        nc.scalar.dma_start(qb1[:], d_qb[H0:H, :])

        # decoder weights (allocated now, DMA'd later to keep the SP DGE ring
        # clear for encT during the attention phase)
        wihc0 = wpool.tile([128, G4], dt.bfloat16)
        wihc1 = wpool.tile([73, G4], dt.bfloat16)
        whh0 = wpool.tile([H0, G4], dt.bfloat16)
        whh1 = wpool.tile([H1, G4], dt.bfloat16)
        wxr_sb = wpool.tile([NB, G4], dt.bfloat16)
        w1t0 = wpool.tile([128, 100], dt.bfloat16)
        w1t1 = wpool.tile([73, 100], dt.bfloat16)
        w2t = wpool.tile([101, 50], dt.bfloat16)
        w3t = wpool.tile([51, 1], dt.bfloat16)
        c0_sb = spool.tile([NB, H], dt.bfloat16)

        # ---------- phase 0: qprojT = Wa @ q^T + (ba + bua) ----------
        qproj0 = spool.tile([H0, NB], dt.float32)  # fp32 bias tiles for tanh
        qproj1 = spool.tile([H1, NB], dt.float32)
        with tc.tile_pool(name="qp_psum", bufs=1, space="PSUM") as qp_ps:
            for m, (msz, qdst, qb) in enumerate(
                [(H0, qproj0, qb0), (H1, qproj1, qb1)]
            ):
                mlo = m * H0
                ps = qp_ps.tile([128, NB], dt.float32, tag="qp")
                nc.tensor.matmul(
                    ps[0:msz, :], wa0[:, mlo : mlo + msz], qt0[:], start=True, stop=False
                )
                nc.tensor.matmul(
                    ps[0:msz, :], wa1[:, mlo : mlo + msz], qt1[:], start=False, stop=True
                )
                nc.vector.tensor_scalar_add(qdst[:], ps[0:msz, :], qb[:])

        # ---------- phase 1: kproj + tanh + scores (flipped) ----------
        # scores land in ONE [128, NB*NCH] PSUM tile: column 16*b + c holds
        # the 128 t's of stride class c (t = 16*k + c -> partition k), which
        # matches the encN "(p n) h" SBUF layout used by the context matmuls.
        encT_pool = ctx.enter_context(tc.tile_pool(name="encT_pool", bufs=3))
        e_pool = ctx.enter_context(tc.tile_pool(name="e_pool", bufs=4))
        encN_pool = ctx.enter_context(tc.tile_pool(name="encN_pool", bufs=16))
        sc_pool = ctx.enter_context(tc.tile_pool(name="sc_psum", bufs=1, space="PSUM"))
        sc = sc_pool.tile([128, NB * NCH], dt.float32, tag="sc")
        p_sb = spool.tile([128, NB * NCH], dt.bfloat16)

        en_tiles = []
        e_saved = []  # (e0, e1) per batch, scores emitted with 1-batch skew

        def emit_scores(b, e0, e1):
            for c in range(NCH):
                col = NB * 0 + b * NCH + c
                nc.tensor.matmul(
                    sc[:, col : col + 1],
                    e0[:, c : T : NCH],
                    va0[:],
                    start=True,
                    stop=False,
                )
                nc.tensor.matmul(
                    sc[:, col : col + 1],
                    e1[:, c : T : NCH],
                    va1[:],
                    start=False,
                    stop=True,
                )

        nc.sync.dma_start(ua_p[:], d_UaT[:, :])
        ua3 = ua_p[:].rearrange("p (i m) -> p i m", i=2)
        with tc.tile_pool(name="kp_psum", bufs=3, space="PSUM") as kp_ps:
            for b in range(NB):
                et = encT_pool.tile([128, 2 * T], dt.float8e4, tag="et")
                nc.sync.dma_start(et[:], d_encT[b, :])
                et3 = et[:].rearrange("p (i t) -> p i t", i=2)
                e0 = e_pool.tile([H0, T], dt.bfloat16, tag="e0")
                e1 = e_pool.tile([H1, T], dt.bfloat16, tag="e1")
                i_kp = None
                for m, (msz, edst, qp) in enumerate(
                    [(H0, e0, qproj0), (H1, e1, qproj1)]
                ):
                    mlo = m * H0
                    for th in range(2):  # 1024-wide psum tiles
                        ps = kp_ps.tile([128, 1024], dt.float32, tag="kp")
                        for n in range(2):
                            c0c = th * 1024 + n * 512
                            i_kp = nc.tensor.matmul(
                                ps[0:msz, n * 512 : (n + 1) * 512],
                                ua3[:, :, mlo : mlo + msz],
                                et3[:, :, c0c : c0c + 512],
                                start=True,
                                stop=True,
                                perf_mode=mybir.MatmulPerfMode.DoubleRow,
                            )
                        # e = tanh(kproj/64 + qproj[:, b]) ; write bf16
                        nc.scalar.activation(
                            edst[:, th * 1024 : (th + 1) * 1024],
                            ps[0:msz, :],
                            AF.Tanh,
                            bias=qp[:, b : b + 1],
                            scale=1.0 / 64.0,
                        )
                # scores with a 1-batch skew so the PE never stalls waiting on
                # this batch's tanh while the next batch's kproj is runnable
                if b > 0:
                    emit_scores(b - 1, *e_saved[b - 1])
                e_saved.append((e0, e1))
                # encN loads paced on the (otherwise idle) SWDGE ring,
                # gated behind this batch's kproj to keep encT priority
                import bass_rust as _br

                en = encN_pool.tile(
                    [128, NCH * H], dt.bfloat16, name=f"en{b}", tag="en"
                )
                i_en = nc.gpsimd.dma_start(
                    en[:],
                    d_encN[b].rearrange("(p n) h -> p (n h)", p=128),
                )
                _br.add_dep_helper(
                    i_en.ins, i_kp.ins, sync=True,
                    reason="encN paced behind this batch's kproj",
                )
                en_tiles.append(en)
            emit_scores(NB - 1, *e_saved[NB - 1])

        # deferred decoder-weight loads (SP ring is now free)
        nc.sync.dma_start(wihc0[:], d_WihcT[0:128, :])
        nc.sync.dma_start(wihc1[:], d_WihcT[128 : H + 1, :])
        nc.sync.dma_start(whh0[:], d_WhhT[0:H0, :])
        nc.sync.dma_start(whh1[:], d_WhhT[H0:H, :])
        nc.sync.dma_start(wxr_sb[:], d_wxr[:, :])
        nc.sync.dma_start(w1t0[:], d_W1T[0:128, :])
        nc.sync.dma_start(w1t1[:], d_W1T[128 : H + 1, :])
        nc.sync.dma_start(w2t[:], d_W2T[:, :])
        nc.sync.dma_start(w3t[:], d_W3T[:, :])
        nc.sync.dma_start(c0_sb[:], d_c0[:, :])

        # ---------- phase 2: softmax pieces ----------
        # p = exp(scores) in one shot (no max-subtraction: |scores| <= ~8).
        # Z_b via two tiny PE reductions; normalization deferred to the ctx
        # scale (context is linear in p).
        ct0 = spool.tile([H0, NB], dt.bfloat16)
        ct1 = spool.tile([H1 + 1, NB], dt.bfloat16)  # row 72 = ones (bias row)
        nc.scalar.dma_start(ct1[H1 : H1 + 1, :], d_ones_b[:, :])
        g0_pool = ctx.enter_context(tc.tile_pool(name="g0_psum", bufs=1, space="PSUM"))
        gp = g0_pool.tile([NB, G4], dt.float32, tag="g0")

        with tc.tile_pool(name="z_psum", bufs=1, space="PSUM") as z_ps:
            for n, nsz in [(0, 512), (512, G4 - 512)]:
                nc.tensor.matmul(
                    gp[:, n : n + nsz], qt0[:], whh0[:, n : n + nsz],
                    start=True, stop=False,
                )
                nc.tensor.matmul(
                    gp[:, n : n + nsz], qt1[:], whh1[:, n : n + nsz],
                    start=False, stop=False,
                )
            nc.scalar.activation(p_sb[:], sc[:], AF.Exp)
            # per-(batch,chunk) partition sums: zc[ch, b]
            zc = z_ps.tile([NCH, NB], dt.float32, tag="zc")
            for b in range(NB):
                nc.tensor.matmul(
                    zc[:, b : b + 1],
                    p_sb[:, b * NCH : (b + 1) * NCH],
                    ones_c_bf[:],
                    start=True,
                    stop=True,
                )
            zc_sb = spool.tile([NCH, NB], dt.float32)
            nc.vector.tensor_copy(zc_sb[:], zc[:])
            # Z per batch as a row [1, NB], then 1/Z, then broadcast to
            # [128, NB] via a K=1 outer product
            zrow = z_ps.tile([1, NB], dt.float32, tag="zrow")
            nc.tensor.matmul(
                zrow[:], ones_c_f[0:NCH, :], zc_sb[:], start=True, stop=True
            )
            rz_sb = spool.tile([1, NB], dt.float32)
            nc.vector.reciprocal(rz_sb[:], zrow[:])
            rzb = z_ps.tile([128, NB], dt.float32, tag="rzb")
            nc.tensor.matmul(rzb[:], ones_r_f[:], rz_sb[:], start=True, stop=True)
            rzb_sb = spool.tile([128, NB], dt.float32)
            nc.vector.tensor_copy(rzb_sb[:], rzb[:])

            # ---------- phase 3: context (flipped: encN stationary, N=1) ----
            ct0_ps = z_ps.tile([H0, NB], dt.float32, tag="ct0")
            ct1_ps = z_ps.tile([H1, NB], dt.float32, tag="ct1")
            for b in range(NB):
                for c in range(NCH):
                    pcol = p_sb[:, b * NCH + c : b * NCH + c + 1]
                    nc.tensor.matmul(
                        ct0_ps[:, b : b + 1],
                        en_tiles[b][:, c * H : c * H + H0],
                        pcol,
                        start=(c == 0),
                        stop=(c == NCH - 1),
                    )
                    nc.tensor.matmul(
                        ct1_ps[:, b : b + 1],
                        en_tiles[b][:, c * H + H0 : (c + 1) * H],
                        pcol,
                        start=(c == 0),
                        stop=(c == NCH - 1),
                    )
            # normalize: ctxT = ctx_raw * (1/Z) broadcast, cast bf16
            nc.vector.tensor_tensor(ct0[:], ct0_ps[:], rzb_sb[:], op=OP.mult)
            nc.vector.tensor_tensor(
                ct1[0:H1, :], ct1_ps[:], rzb_sb[0:H1, :], op=OP.mult
            )
            for n, nsz in [(0, 512), (512, G4 - 512)]:
                nc.tensor.matmul(
                    gp[:, n : n + nsz], ct0[:], wihc0[:, n : n + nsz],
                    start=False, stop=False,
                )
                nc.tensor.matmul(
                    gp[:, n : n + nsz], ct1[:], wihc1[:, n : n + nsz],
                    start=False, stop=True,
                )


        # ---------- phase 5: decoder steps (all bf16, gate order f,i,o|g) ---
        x_sb = spool.tile([NB, 1], dt.float32)
        nc.sync.dma_start(x_sb[:], d_x0[:, :])
        xn_all = spool.tile([NB, NSTEPS], dt.float32)
        ht0 = spool.tile([H0, NB], dt.bfloat16)
        ht1 = spool.tile([H1 + 1, NB], dt.bfloat16)  # row 72 = ones (b1 row)
        nc.scalar.dma_start(ht1[H1 : H1 + 1, :], d_ones_b[:, :])
        o1t = spool.tile([101, NB], dt.bfloat16)  # row 100 = ones (b2 row)
        nc.scalar.dma_start(o1t[100:101, :], d_ones_b[:, :])
        o2t = spool.tile([51, NB], dt.bfloat16)  # row 50 = ones (b3 row)
        nc.scalar.dma_start(o2t[50:51, :], d_ones_b[:, :])

        with (
            tc.tile_pool(name="ls", bufs=2) as ls,
            tc.tile_pool(name="ls_psum", bufs=3, space="PSUM") as lp,
            tc.tile_pool(name="m3_psum", bufs=2, space="PSUM") as mp,
        ):
            xt = x_sb
            for t in range(NSTEPS):
                # gates = g0 + x * wxr, fused on DVE; split f,i,o vs g so
                # the sigmoid can start before the g slice is computed
                gates = ls.tile([NB, G4], dt.bfloat16, tag="gates")
                nc.vector.scalar_tensor_tensor(
                    gates[:, 0 : 3 * H], wxr_sb[:, 0 : 3 * H], xt[:, 0:1],
                    gp[:, 0 : 3 * H], op0=OP.mult, op1=OP.add,
                )
                nc.vector.scalar_tensor_tensor(
                    gates[:, 3 * H : 4 * H], wxr_sb[:, 3 * H : 4 * H], xt[:, 0:1],
                    gp[:, 3 * H : 4 * H], op0=OP.mult, op1=OP.add,
                )
                # f,i,o sigmoid in one instr; g tanh
                sfio = ls.tile([NB, 3 * H], dt.bfloat16, tag="sfio")
                nc.scalar.activation(sfio[:], gates[:, 0 : 3 * H], AF.Sigmoid)
                tg = ls.tile([NB, H], dt.bfloat16, tag="tg")
                nc.scalar.activation(tg[:], gates[:, 3 * H : 4 * H], AF.Tanh)
                t1 = ls.tile([NB, H], dt.bfloat16, tag="t1")
                nc.vector.tensor_tensor(t1[:], sfio[:, 0:H], c0_sb[:], op=OP.mult)
                t2 = ls.tile([NB, H], dt.bfloat16, tag="t2")
                nc.vector.tensor_tensor(t2[:], sfio[:, H : 2 * H], tg[:], op=OP.mult)
                cn = ls.tile([NB, H], dt.bfloat16, tag="cn")
                nc.vector.tensor_tensor(cn[:], t1[:], t2[:], op=OP.add)
                tcn = ls.tile([NB, H], dt.bfloat16, tag="tcn")
                nc.scalar.activation(tcn[:], cn[:], AF.Tanh)
                hh = ls.tile([NB, H], dt.bfloat16, tag="hh")
                nc.vector.tensor_tensor(hh[:], sfio[:, 2 * H : 3 * H], tcn[:], op=OP.mult)
                # transpose h -> ht0/ht1 (feature-major for the MLP); relu
                # folded into the PSUM->SBUF max-copies on DVE
                tp0 = lp.tile([128, NB], dt.bfloat16, tag="lsps")
                nc.tensor.transpose(tp0[:], hh[:, 0:H0], id_bf[0:NB, 0:NB])
                nc.vector.tensor_scalar_max(ht0[:], tp0[:], 0.0)
                tp1 = lp.tile([128, NB], dt.bfloat16, tag="lsps")
                nc.tensor.transpose(tp1[0:H1, :], hh[:, H0:H], id_bf[0:NB, 0:NB])
                nc.scalar.activation(ht1[0:H1, :], tp1[0:H1, :], AF.Relu)
                # MLP in feature-major, biases via ones rows
                m1 = lp.tile([100, NB], dt.float32, tag="lsps")
                nc.tensor.matmul(m1[:], w1t0[:], ht0[:], start=True, stop=False)
                nc.tensor.matmul(m1[:], w1t1[:], ht1[:], start=False, stop=True)
                nc.vector.tensor_scalar_max(o1t[0:100, :], m1[:], 0.0)
                m2 = lp.tile([50, NB], dt.float32, tag="lsps")
                nc.tensor.matmul(m2[:], w2t[:], o1t[:], start=True, stop=True)
                nc.vector.tensor_scalar_max(o2t[0:50, :], m2[:], 0.0)
                # m3 flipped: o2 stationary, w3 moving -> out [NB, 1] is
                # directly the next step's x (and this step's y)
                m3 = mp.tile([NB, 1], dt.float32, tag="m3")
                nc.tensor.matmul(m3[:], o2t[:], w3t[:], start=True, stop=True)
                nc.vector.tensor_copy(xn_all[:, t : t + 1], m3[:])
                xt = m3
            nc.sync.dma_start(d_y[:, :], xn_all[:])

    # Bacc lowering: register allocation + wait splitting (<=1 wait/inst on HW)
    nc.compile()
    return nc


def _prep_inputs(x, h0, c0, encoder_output, Wa, ba, Ua, bua, Va, bva,
                 W_ih, W_hh, b_ih, b_hh, W1, b1, W2, b2, W3, b3):
    """Host-side layout prep -> list of per-core input maps."""
    f32 = np.float32
    enc = np.ascontiguousarray(encoder_output, dtype=f32)
    q = np.asarray(h0, dtype=f32)[0]          # [B, H]
    c0f = np.asarray(c0, dtype=f32)[0]        # [B, H]
    x0 = np.asarray(x, dtype=f32).reshape(B, 1)

    # gate permutation: torch order (i,f,g,o) -> device order (f,i,o,g) so
    # one sigmoid instr covers f,i,o and tanh covers g
    gperm = np.concatenate(
        [np.arange(H, 2 * H), np.arange(0, H), np.arange(3 * H, 4 * H),
         np.arange(2 * H, 3 * H)]
    )
    W_ihp = np.asarray(W_ih, f32)[gperm]
    W_hhp = np.asarray(W_hh, f32)[gperm]
    bp = (np.asarray(b_ih, f32) + np.asarray(b_hh, f32))[gperm]

    FP8 = ml_dtypes.float8_e4m3fn
    # Ua scaled x64 into fp8 comfortable range; kernel rescales inside tanh.
    # DoubleRow packing: partition p holds h_in = p (i=0) and 100+p (i=1).
    uaT = np.asarray(Ua, f32).T * 64.0  # [h_in, h_out]
    uaT_pad = np.zeros((256, 208), f32)
    uaT_pad[0:H, 0:H] = uaT
    uaT_p = np.stack([uaT_pad[0:128], uaT_pad[128:256]], axis=1)  # [128, 2, 208]

    # replicated weights (shared by every core)
    shared = {
        "UaTp": np.ascontiguousarray(uaT_p.reshape(128, 2 * 208)).astype(FP8),
        "WaT": np.ascontiguousarray(np.asarray(Wa, f32).T).astype(BF16),
        "qb": (np.asarray(ba, f32) + np.asarray(bua, f32)).reshape(H, 1),
        "VaT": np.ascontiguousarray(np.asarray(Va, f32)[0].reshape(H, 1)).astype(BF16),
        "WihcT": np.concatenate(
            [W_ihp[:, 1:].T, bp.reshape(1, G4)], axis=0
        ).astype(BF16),
        "WhhT": np.ascontiguousarray(W_hhp.T).astype(BF16),
        "wxr": np.broadcast_to(W_ihp[:, 0].reshape(1, G4), (NB, G4)).astype(BF16),
        "W1T": np.concatenate(
            [np.asarray(W1, f32).T, np.asarray(b1, f32).reshape(1, 100)], axis=0
        ).astype(BF16),
        "W2T": np.concatenate(
            [np.asarray(W2, f32).T, np.asarray(b2, f32).reshape(1, 50)], axis=0
        ).astype(BF16),
        "W3T": np.concatenate(
            [np.asarray(W3, f32).T, np.asarray(b3, f32).reshape(1, 1)], axis=0
        ).astype(BF16),
        "ones_b": np.ones((1, NB), BF16),
    }

    in_maps = []
    for c in range(NCORES):
        bs = slice(c * NB, (c + 1) * NB)
        enc_c = enc[bs]  # [NB, T, H]
        m = dict(shared)
        encT_c = enc_c.transpose(0, 2, 1)  # [NB, H, T]
        encT_pad = np.concatenate(
            [encT_c, np.zeros((NB, 56, T), f32)], axis=1
        )  # [NB, 256, T]
        encT_p = np.stack([encT_pad[:, 0:128], encT_pad[:, 128:256]], axis=2)
        m["encTp"] = np.ascontiguousarray(encT_p.reshape(NB, 128, 2 * T)).astype(FP8)
        m["encN"] = enc_c.astype(BF16)
        m["qT"] = np.ascontiguousarray(q[bs].T).astype(BF16)
        m["c0s"] = np.ascontiguousarray(c0f[bs]).astype(BF16)
        m["x0s"] = np.ascontiguousarray(x0[bs])
        in_maps.append(m)
    return in_maps


def kernel(**inputs):
    from concourse.bass_utils import run_bass_kernel_spmd

    if "nc" not in _CACHE:
        _CACHE["nc"] = _build_module()
    nc = _CACHE["nc"]

    in_maps = _prep_inputs(**inputs)
    res = run_bass_kernel_spmd(nc, in_maps, core_ids=list(range(NCORES)))
    # y2 per core: [NB, NSTEPS] -> full output [B, NSTEPS]
    out = np.concatenate([r["y2"] for r in res.results], axis=0)
    return np.ascontiguousarray(out.astype(np.float32))


# revision 16
# speedup vs baseline: 1.6212x; 1.0451x over previous
"""Trainium2 Bass kernel for nn_DecoderAttention (Bahdanau attention + LSTM decoder).

Data-parallel over batch: B=128 split across 8 NeuronCores (16 batches/core).
All FLOPs run on device; the host only reshuffles layouts (transpose / dtype
cast / fp8 DoubleRow packing / weight concat with bias rows folded in).

Per-core device pipeline (cost-model-aware layout):
  phase 0: ONE packed DMA for all small attention weights (+one fp32 qb DMA),
           qprojT = Wa @ q^T (+ ba + bua) on PE
  phase 1: per batch b: kprojT = Ua @ enc_b^T as fp8 DoubleRow matmuls
           (K=200 packed 2/partition, one pass, 0.5 cyc/row),
           tanh(kproj/64 + qprojT[:, b]) on ACT -> e tiles [h, t] bf16;
           scores as FLIPPED matmuls: e chunk stationary (K=h, M=128 t's of
           one stride-16 class), Va moving (N=1) -> scores accumulate into
           one [128, 256] PSUM tile, column 16*b+c;
           per WAVE of 4 batches: exp slice [128, 64] -> p (bf16,
           unnormalized), per-batch colsum matmuls -> zc, context via FLIPPED
           matmuls (encN chunk stationary K=t, p column moving N=1) -> ctxT
           accumulates [h, b] in PSUM. All of it hides inside phase 1.
  phase 2 (tail): Z totals via two tiny PE reductions, 1/Z broadcast via a
           K=1 outer-product matmul, ctxT scale on DVE, G0 closes an
           accumulation whose q-terms ran during phase 1.
  phase 3: 5 serial decoder steps, all-bf16 elementwise:
           gates = G0(PSUM) + x*wxr fused on DVE (scalar_tensor_tensor,
           split f,i,o vs g), gate order permuted so one sigmoid covers
           f,i,o; MLP in bf16; m3 flipped so x_next = out [16, 1] feeds the
           next step's scalar directly from PSUM.
"""

import numpy as np
import ml_dtypes

B, T, H = 128, 2048, 200
NCORES = 8
NB = B // NCORES  # 16 batches per core
NSTEPS = 5
G4 = 4 * H  # 800 gate width
NCH = T // 128  # 16 stride-class chunks (t = 16*k + c -> partition k, chunk c)

_CACHE = {}

BF16 = ml_dtypes.bfloat16
FP8 = ml_dtypes.float8_e4m3fn

# packed attention-weight tensor column layout (bf16, [128, PK_C])
PK_WA0, PK_WA1 = 0, 200          # wa0 [128,200] | wa1 [72,200]
PK_VA0, PK_VA1 = 400, 401        # va columns
PK_QT0, PK_QT1 = 402, 418        # qt [*,16]
PK_C = 434

# packed decoder-weight tensor column layout (bf16, [128, DK_C])
DK_WIHC0, DK_WIHC1 = 0, 800      # wihc0 [128,800] | wihc1 [73,800] (row72=bias)
DK_WHH0, DK_WHH1 = 1600, 2400    # whh [128|72, 800]
DK_WXR = 3200                    # wxr [16, 800]
DK_W1T0, DK_W1T1 = 4000, 4100    # w1t [128|73, 100] (row72=b1)
DK_W2T = 4200                    # w2t [101, 50] (row100=b2)
DK_W3T = 4250                    # w3t [51, 1] (row50=b3)
DK_C0 = 4251                     # c0 [16, 200]
DK_C = 4451


def _build_module():
    """Build the Bass module (same NEFF for all 8 cores)."""
    from contextlib import ExitStack

    import concourse.bass as bass
    import concourse.tile as tile
    from concourse import bacc, mybir
    from concourse.masks import make_identity

    dt = mybir.dt
    AF = mybir.ActivationFunctionType
    OP = mybir.AluOpType

    nc = bacc.Bacc(
        "TRN2",
        target_bir_lowering=False,
        debug=False,
        num_devices=NCORES,
    )

    # ---- DRAM tensors (per-core shards; weights replicated) ----
    d_encT = nc.dram_tensor(
        "encTp", [NB, 128, 2 * T], dt.float8e4, kind="ExternalInput"
    ).ap()  # DoubleRow packing: col i*T+t, partition p <-> h_in = i*128+p
    d_encN = nc.dram_tensor("encN", [NB, T, H], dt.bfloat16, kind="ExternalInput").ap()
    d_x0 = nc.dram_tensor("x0s", [NB, 1], dt.float32, kind="ExternalInput").ap()
    d_UaT = nc.dram_tensor("UaTp", [128, 2 * 208], dt.float8e4, kind="ExternalInput").ap()
    d_wpk = nc.dram_tensor("wpk", [128, PK_C], dt.bfloat16, kind="ExternalInput").ap()
    d_qb = nc.dram_tensor("qb2", [128, 2], dt.float32, kind="ExternalInput").ap()
    d_dpk = nc.dram_tensor("dpk", [128, DK_C], dt.bfloat16, kind="ExternalInput").ap()
    d_ones_b = nc.dram_tensor("ones_b", [1, NB], dt.bfloat16, kind="ExternalInput").ap()
    d_y = nc.dram_tensor("y2", [NB, NSTEPS], dt.float32, kind="ExternalOutput").ap()

    H0, H1 = 128, H - 128  # 128 + 72 partition chunks of the hidden dim

    with tile.TileContext(nc) as tc, ExitStack() as ctx:
        # ---------- persistent pools ----------
        wpool = ctx.enter_context(tc.tile_pool(name="weights", bufs=1))
        spool = ctx.enter_context(tc.tile_pool(name="smalls", bufs=1))

        # warmup: preload the tanh/exp ACT table set while DMAs stream
        wt_a = spool.tile([1, 8], dt.float32)
        nc.vector.memset(wt_a[:], 0.0)
        wt_b = spool.tile([1, 8], dt.float32)
        nc.scalar.activation(wt_b[:], wt_a[:], AF.Tanh)

        # identity for the decoder's h transposes (bf16)
        id_bf = wpool.tile([128, 128], dt.bfloat16)
        make_identity(nc, id_bf[:])

        # ones columns/rows for the tiny PE reductions (sliced on read)
        ones_c_bf = wpool.tile([128, 1], dt.bfloat16)
        nc.vector.memset(ones_c_bf[:], 1.0)
        ones_c_f = wpool.tile([128, 1], dt.float32)
        nc.vector.memset(ones_c_f[:], 1.0)
        ones_r_f = wpool.tile([1, 128], dt.float32)
        nc.vector.memset(ones_r_f[:], 1.0)

        # packed attention weights: one DMA instead of ten
        wpk = wpool.tile([128, PK_C], dt.bfloat16)
        nc.scalar.dma_start(wpk[:], d_wpk[:, :])
        qb2 = wpool.tile([128, 2], dt.float32)
        nc.scalar.dma_start(qb2[:], d_qb[:, :])
        wa0 = wpk[:, PK_WA0 : PK_WA0 + 200]
        wa1 = wpk[0:H1, PK_WA1 : PK_WA1 + 200]
        va0 = wpk[:, PK_VA0 : PK_VA0 + 1]
        va1 = wpk[0:H1, PK_VA1 : PK_VA1 + 1]
        qt0 = wpk[:, PK_QT0 : PK_QT0 + NB]
        qt1 = wpk[0:H1, PK_QT1 : PK_QT1 + NB]

        ua_p = wpool.tile([128, 2 * 208], dt.float8e4)
        nc.sync.dma_start(ua_p[:], d_UaT[:, :])
        ua3 = ua_p[:].rearrange("p (i m) -> p i m", i=2)

        # packed decoder weights: one DMA (deferred below, behind first encT)
        dpk = wpool.tile([128, DK_C], dt.bfloat16)
        wihc0 = dpk[:, DK_WIHC0 : DK_WIHC0 + G4]
        wihc1 = dpk[0 : H1 + 1, DK_WIHC1 : DK_WIHC1 + G4]
        whh0 = dpk[:, DK_WHH0 : DK_WHH0 + G4]
        whh1 = dpk[0:H1, DK_WHH1 : DK_WHH1 + G4]
        wxr_sb = dpk[0:NB, DK_WXR : DK_WXR + G4]
        w1t0 = dpk[:, DK_W1T0 : DK_W1T0 + 100]
        w1t1 = dpk[0 : H1 + 1, DK_W1T1 : DK_W1T1 + 100]
        w2t = dpk[0:101, DK_W2T : DK_W2T + 50]
        w3t = dpk[0:51, DK_W3T : DK_W3T + 1]
        c0_sb = dpk[0:NB, DK_C0 : DK_C0 + H]

        # ---------- phase 0: qprojT = Wa @ q^T + (ba + bua) ----------
        qproj0 = spool.tile([H0, NB], dt.float32)  # fp32 bias tiles for tanh
        qproj1 = spool.tile([H1, NB], dt.float32)
        with tc.tile_pool(name="qp_psum", bufs=1, space="PSUM") as qp_ps:
            for m, (msz, qdst) in enumerate([(H0, qproj0), (H1, qproj1)]):
                mlo = m * H0
                ps = qp_ps.tile([128, NB], dt.float32, tag="qp")
                nc.tensor.matmul(
                    ps[0:msz, :], wa0[:, mlo : mlo + msz], qt0, start=True, stop=False
                )
                nc.tensor.matmul(
                    ps[0:msz, :], wa1[:, mlo : mlo + msz], qt1, start=False, stop=True
                )
                nc.vector.tensor_scalar_add(
                    qdst[:], ps[0:msz, :], qb2[0:msz, m : m + 1]
                )

        # G0 PSUM lives from phase 2 through the decoder; allocate its pool
        # ahead of the attention pools so releases stay LIFO
        g0_pool = ctx.enter_context(tc.tile_pool(name="g0_psum", bufs=1, space="PSUM"))
        gp = g0_pool.tile([NB, G4], dt.float32, tag="g0")

        # ---------- phase 1: kproj + tanh + scores + per-wave softmax/ctx ----
        encT_pool = ctx.enter_context(tc.tile_pool(name="encT_pool", bufs=3))
        e_pool = ctx.enter_context(tc.tile_pool(name="e_pool", bufs=4))
        encN_pool = ctx.enter_context(tc.tile_pool(name="encN_pool", bufs=16))
        from contextlib import ExitStack as _ES
        att_ctx = _ES()
        sc_pool = att_ctx.enter_context(tc.tile_pool(name="sc_psum", bufs=1, space="PSUM"))
        ct_pool = att_ctx.enter_context(tc.tile_pool(name="ct_psum", bufs=1, space="PSUM"))
        sc = sc_pool.tile([128, NB * NCH], dt.float32, tag="sc")
        p_sb = spool.tile([128, NB * NCH], dt.bfloat16)
        # one PSUM tile (PSUM tiles are bank-granular): ct0 | ct1 | zc columns
        ctz = ct_pool.tile([128, 3 * NB], dt.float32, tag="ctz")
        ct0_ps = ctz[:, 0:NB]
        ct1_ps = ctz[0:H1, NB : 2 * NB]
        zc = ctz[0:NCH, 2 * NB : 3 * NB]

        en_tiles = []
        e_saved = []  # (e0, e1) per batch; scores emitted with 1-batch skew

        def emit_scores(b):
            e0, e1 = e_saved[b]
            for c in range(NCH):
                col = b * NCH + c
                nc.tensor.matmul(
                    sc[:, col : col + 1], e0[:, c : T : NCH], va0,
                    start=True, stop=False,
                )
                nc.tensor.matmul(
                    sc[:, col : col + 1], e1[:, c : T : NCH], va1,
                    start=False, stop=True,
                )

        def emit_wave(w):
            # exp + Z colsums + context for batches 4w..4w+3 (scores ready)
            lo = 4 * w * NCH
            nc.scalar.activation(
                p_sb[:, lo : lo + 4 * NCH], sc[:, lo : lo + 4 * NCH], AF.Exp
            )
            for b in range(4 * w, 4 * w + 4):
                nc.tensor.matmul(
                    zc[:, b : b + 1],
                    p_sb[:, b * NCH : (b + 1) * NCH],
                    ones_c_bf[:],
                    start=True,
                    stop=True,
                )
                for c in range(NCH):
                    pcol = p_sb[:, b * NCH + c : b * NCH + c + 1]
                    nc.tensor.matmul(
                        ct0_ps[:, b : b + 1],
                        en_tiles[b][:, c * H : c * H + H0],
                        pcol,
                        start=(c == 0),
                        stop=(c == NCH - 1),
                    )
                    nc.tensor.matmul(
                        ct1_ps[:, b : b + 1],
                        en_tiles[b][:, c * H + H0 : (c + 1) * H],
                        pcol,
                        start=(c == 0),
                        stop=(c == NCH - 1),
                    )

        with tc.tile_pool(name="kp_psum", bufs=2, space="PSUM") as kp_ps:
            for b in range(NB):
                et = encT_pool.tile([128, 2 * T], dt.float8e4, tag="et")
                nc.sync.dma_start(et[:], d_encT[b, :])
                et3 = et[:].rearrange("p (i t) -> p i t", i=2)
                if b == 1:
                    # single packed decoder-weight DMA, behind the first encT
                    nc.sync.dma_start(dpk[:], d_dpk[:, :])
                e0 = e_pool.tile([H0, T], dt.bfloat16, tag="e0")
                e1 = e_pool.tile([H1, T], dt.bfloat16, tag="e1")
                i_kp = None
                for m, (msz, edst, qp) in enumerate(
                    [(H0, e0, qproj0), (H1, e1, qproj1)]
                ):
                    mlo = m * H0
                    for th in range(2):  # 1024-wide psum tiles
                        ps = kp_ps.tile([128, 1024], dt.float32, tag="kp")
                        for n in range(2):
                            c0c = th * 1024 + n * 512
                            i_kp = nc.tensor.matmul(
                                ps[0:msz, n * 512 : (n + 1) * 512],
                                ua3[:, :, mlo : mlo + msz],
                                et3[:, :, c0c : c0c + 512],
                                start=True,
                                stop=True,
                                perf_mode=mybir.MatmulPerfMode.DoubleRow,
                            )
                        # e = tanh(kproj/64 + qproj[:, b]) ; write bf16
                        nc.scalar.activation(
                            edst[:, th * 1024 : (th + 1) * 1024],
                            ps[0:msz, :],
                            AF.Tanh,
                            bias=qp[:, b : b + 1],
                            scale=1.0 / 64.0,
                        )
                # scores with a 1-batch skew; waves fire as they complete
                if b > 0:
                    emit_scores(b - 1)
                    if b % 4 == 0:
                        emit_wave(b // 4 - 1)
                e_saved.append((e0, e1))
                # encN loads paced on the (otherwise idle) SWDGE ring,
                # gated behind this batch's kproj to keep encT priority
                import bass_rust as _br

                en = encN_pool.tile(
                    [128, NCH * H], dt.bfloat16, name=f"en{b}", tag="en"
                )
                i_en = nc.gpsimd.dma_start(
                    en[:],
                    d_encN[b].rearrange("(p n) h -> p (n h)", p=128),
                )
                _br.add_dep_helper(
                    i_en.ins, i_kp.ins, sync=True,
                    reason="encN paced behind this batch's kproj",
                )
                en_tiles.append(en)
            emit_scores(NB - 1)
            emit_wave(3)

        # ---------- phase 2: Z totals, 1/Z, ctx scale, G0 ----------
        ct0 = spool.tile([H0, NB], dt.bfloat16)
        ct1 = spool.tile([H1 + 1, NB], dt.bfloat16)  # row 72 = ones (bias row)
        nc.scalar.dma_start(ct1[H1 : H1 + 1, :], d_ones_b[:, :])

        with tc.tile_pool(name="z_psum", bufs=1, space="PSUM") as z_ps:
            # q-dependent G0 terms: no ctx dependency, run right away
            for n, nsz in [(0, 512), (512, G4 - 512)]:
                nc.tensor.matmul(
                    gp[:, n : n + nsz], qt0, whh0[:, n : n + nsz],
                    start=True, stop=False,
                )
                nc.tensor.matmul(
                    gp[:, n : n + nsz], qt1, whh1[:, n : n + nsz],
                    start=False, stop=False,
                )
            # Z per batch as a row [1, NB], then 1/Z, broadcast to [128, NB]
            zc_sb = spool.tile([NCH, NB], dt.float32)
            nc.vector.tensor_copy(zc_sb[:], zc[:])
            zrow = z_ps.tile([1, NB], dt.float32, tag="zrow")
            nc.tensor.matmul(
                zrow[:], ones_c_f[0:NCH, :], zc_sb[:], start=True, stop=True
            )
            rz_sb = spool.tile([1, NB], dt.float32)
            nc.vector.reciprocal(rz_sb[:], zrow[:])
            rzb = z_ps.tile([128, NB], dt.float32, tag="rzb")
            nc.tensor.matmul(rzb[:], ones_r_f[:], rz_sb[:], start=True, stop=True)
            rzb_sb = spool.tile([128, NB], dt.float32)
            nc.vector.tensor_copy(rzb_sb[:], rzb[:])
            # normalize: ctxT = ctx_raw * (1/Z) broadcast, cast bf16
            nc.vector.tensor_tensor(ct0[:], ct0_ps[:], rzb_sb[:], op=OP.mult)
            nc.vector.tensor_tensor(
                ct1[0:H1, :], ct1_ps[:], rzb_sb[0:H1, :], op=OP.mult
            )
            for n, nsz in [(0, 512), (512, G4 - 512)]:
                nc.tensor.matmul(
                    gp[:, n : n + nsz], ct0[:], wihc0[:, n : n + nsz],
                    start=False, stop=False,
                )
                nc.tensor.matmul(
                    gp[:, n : n + nsz], ct1, wihc1[:, n : n + nsz],
                    start=False, stop=True,
                )
        att_ctx.close()  # release sc/ct PSUM banks before the decoder

        # ---------- phase 3: decoder steps (all bf16, gate order f,i,o|g) ---
        x_sb = spool.tile([NB, 1], dt.float32)
        nc.sync.dma_start(x_sb[:], d_x0[:, :])
        xn_all = spool.tile([NB, NSTEPS], dt.float32)
        ht0 = spool.tile([H0, NB], dt.bfloat16)
        ht1 = spool.tile([H1 + 1, NB], dt.bfloat16)  # row 72 = ones (b1 row)
        nc.scalar.dma_start(ht1[H1 : H1 + 1, :], d_ones_b[:, :])
        o1t = spool.tile([101, NB], dt.bfloat16)  # row 100 = ones (b2 row)
        nc.scalar.dma_start(o1t[100:101, :], d_ones_b[:, :])
        o2t = spool.tile([51, NB], dt.bfloat16)  # row 50 = ones (b3 row)
        nc.scalar.dma_start(o2t[50:51, :], d_ones_b[:, :])

        with (
            tc.tile_pool(name="ls", bufs=2) as ls,
            tc.tile_pool(name="ls_psum", bufs=3, space="PSUM") as lp,
            tc.tile_pool(name="m3_psum", bufs=2, space="PSUM") as mp,
        ):
            xt = x_sb
            for t in range(NSTEPS):
                # gates = g0 + x * wxr, fused on DVE; split f,i,o vs g so
                # the sigmoid can start before the g slice is computed
                gates = ls.tile([NB, G4], dt.bfloat16, tag="gates")
                nc.vector.scalar_tensor_tensor(
                    gates[:, 0 : 3 * H], wxr_sb[:, 0 : 3 * H], xt[:, 0:1],
                    gp[:, 0 : 3 * H], op0=OP.mult, op1=OP.add,
                )
                nc.vector.scalar_tensor_tensor(
                    gates[:, 3 * H : 4 * H], wxr_sb[:, 3 * H : 4 * H], xt[:, 0:1],
                    gp[:, 3 * H : 4 * H], op0=OP.mult, op1=OP.add,
                )
                # f,i,o sigmoid in one instr; g tanh
                sfio = ls.tile([NB, 3 * H], dt.bfloat16, tag="sfio")
                nc.scalar.activation(sfio[:], gates[:, 0 : 3 * H], AF.Sigmoid)
                tg = ls.tile([NB, H], dt.bfloat16, tag="tg")
                nc.scalar.activation(tg[:], gates[:, 3 * H : 4 * H], AF.Tanh)
                t1 = ls.tile([NB, H], dt.bfloat16, tag="t1")
                nc.vector.tensor_tensor(t1[:], sfio[:, 0:H], c0_sb, op=OP.mult)
                t2 = ls.tile([NB, H], dt.bfloat16, tag="t2")
                nc.vector.tensor_tensor(t2[:], sfio[:, H : 2 * H], tg[:], op=OP.mult)
                cn = ls.tile([NB, H], dt.bfloat16, tag="cn")
                nc.vector.tensor_tensor(cn[:], t1[:], t2[:], op=OP.add)
                tcn = ls.tile([NB, H], dt.bfloat16, tag="tcn")
                nc.scalar.activation(tcn[:], cn[:], AF.Tanh)
                hh = ls.tile([NB, H], dt.bfloat16, tag="hh")
                nc.vector.tensor_tensor(hh[:], sfio[:, 2 * H : 3 * H], tcn[:], op=OP.mult)
                # transpose h -> ht0/ht1 (feature-major for the MLP); relu
                # folded into the PSUM->SBUF copies (DVE max / ACT relu)
                tp0 = lp.tile([128, NB], dt.bfloat16, tag="lsps")
                nc.tensor.transpose(tp0[:], hh[:, 0:H0], id_bf[0:NB, 0:NB])
                nc.vector.tensor_scalar_max(ht0[:], tp0[:], 0.0)
                tp1 = lp.tile([128, NB], dt.bfloat16, tag="lsps")
                nc.tensor.transpose(tp1[0:H1, :], hh[:, H0:H], id_bf[0:NB, 0:NB])
                nc.scalar.activation(ht1[0:H1, :], tp1[0:H1, :], AF.Relu)
                # MLP in feature-major, biases via ones rows
                m1 = lp.tile([100, NB], dt.float32, tag="lsps")
                nc.tensor.matmul(m1[:], w1t0, ht0[:], start=True, stop=False)
                nc.tensor.matmul(m1[:], w1t1, ht1[:], start=False, stop=True)
                nc.vector.tensor_scalar_max(o1t[0:100, :], m1[:], 0.0)
                m2 = lp.tile([50, NB], dt.float32, tag="lsps")
                nc.tensor.matmul(m2[:], w2t, o1t[:], start=True, stop=True)
                nc.vector.tensor_scalar_max(o2t[0:50, :], m2[:], 0.0)
                # m3 flipped: o2 stationary, w3 moving -> out [NB, 1] is
                # directly the next step's x (read from PSUM as STT scalar)
                m3 = mp.tile([NB, 1], dt.float32, tag="m3")
                nc.tensor.matmul(m3[:], o2t[:], w3t, start=True, stop=True)
                nc.vector.tensor_copy(xn_all[:, t : t + 1], m3[:])
                xt = m3
            nc.sync.dma_start(d_y[:, :], xn_all[:])

    # Bacc lowering: register allocation + wait splitting (<=1 wait/inst on HW)
    nc.compile()
    return nc


def _prep_inputs(x, h0, c0, encoder_output, Wa, ba, Ua, bua, Va, bva,
                 W_ih, W_hh, b_ih, b_hh, W1, b1, W2, b2, W3, b3):
    """Host-side layout prep -> list of per-core input maps."""
    f32 = np.float32
    enc = np.ascontiguousarray(encoder_output, dtype=f32)
    q = np.asarray(h0, dtype=f32)[0]          # [B, H]
    c0f = np.asarray(c0, dtype=f32)[0]        # [B, H]
    x0 = np.asarray(x, dtype=f32).reshape(B, 1)

    # gate permutation: torch order (i,f,g,o) -> device order (f,i,o,g) so
    # one sigmoid instr covers f,i,o and tanh covers g
    gperm = np.concatenate(
        [np.arange(H, 2 * H), np.arange(0, H), np.arange(3 * H, 4 * H),
         np.arange(2 * H, 3 * H)]
    )
    W_ihp = np.asarray(W_ih, f32)[gperm]
    W_hhp = np.asarray(W_hh, f32)[gperm]
    bp = (np.asarray(b_ih, f32) + np.asarray(b_hh, f32))[gperm]

    # Ua scaled x64 into fp8 comfortable range; kernel rescales inside tanh.
    # DoubleRow packing: partition p holds h_in = p (i=0) and 128+p (i=1);
    # M padded 200->208 so the pair stride is 16B-aligned.
    uaT = np.asarray(Ua, f32).T * 64.0  # [h_in, h_out]
    uaT_pad = np.zeros((256, 208), f32)
    uaT_pad[0:H, 0:H] = uaT
    uaT_p = np.stack([uaT_pad[0:128], uaT_pad[128:256]], axis=1)  # [128, 2, 208]

    # packed attention weights
    wpk = np.zeros((128, PK_C), f32)
    waT = np.asarray(Wa, f32).T  # [h_in, h_out]
    wpk[:, PK_WA0 : PK_WA0 + 200] = waT[0:128]
    wpk[0:72, PK_WA1 : PK_WA1 + 200] = waT[128:200]
    va = np.asarray(Va, f32)[0]
    wpk[:, PK_VA0] = va[0:128]
    wpk[0:72, PK_VA1] = va[128:200]
    qb = np.asarray(ba, f32) + np.asarray(bua, f32)
    qb2 = np.zeros((128, 2), f32)
    qb2[:, 0] = qb[0:128]
    qb2[0:72, 1] = qb[128:200]

    # packed decoder weights
    dpk = np.zeros((128, DK_C), f32)
    w_ihcT = W_ihp[:, 1:].T  # [H, G4]
    dpk[:, DK_WIHC0 : DK_WIHC0 + G4] = w_ihcT[0:128]
    dpk[0:72, DK_WIHC1 : DK_WIHC1 + G4] = w_ihcT[128:200]
    dpk[72, DK_WIHC1 : DK_WIHC1 + G4] = bp
    w_hhT = W_hhp.T
    dpk[:, DK_WHH0 : DK_WHH0 + G4] = w_hhT[0:128]
    dpk[0:72, DK_WHH1 : DK_WHH1 + G4] = w_hhT[128:200]
    dpk[0:NB, DK_WXR : DK_WXR + G4] = np.broadcast_to(
        W_ihp[:, 0].reshape(1, G4), (NB, G4)
    )
    w1T = np.asarray(W1, f32).T
    dpk[:, DK_W1T0 : DK_W1T0 + 100] = w1T[0:128]
    dpk[0:72, DK_W1T1 : DK_W1T1 + 100] = w1T[128:200]
    dpk[72, DK_W1T1 : DK_W1T1 + 100] = np.asarray(b1, f32)
    dpk[0:100, DK_W2T : DK_W2T + 50] = np.asarray(W2, f32).T
    dpk[100, DK_W2T : DK_W2T + 50] = np.asarray(b2, f32)
    dpk[0:50, DK_W3T] = np.asarray(W3, f32)[0]
    dpk[50, DK_W3T] = np.asarray(b3, f32)[0]

    shared = {
        "UaTp": np.ascontiguousarray(uaT_p.reshape(128, 2 * 208)).astype(FP8),
        "qb2": qb2,
        "ones_b": np.ones((1, NB), BF16),
    }

    in_maps = []
    for cix in range(NCORES):
        bs = slice(cix * NB, (cix + 1) * NB)
        enc_c = enc[bs]  # [NB, T, H]
        m = dict(shared)
        encT_c = enc_c.transpose(0, 2, 1)  # [NB, H, T]
        encT_pad = np.concatenate(
            [encT_c, np.zeros((NB, 56, T), f32)], axis=1
        )  # [NB, 256, T]
        encT_p = np.stack([encT_pad[:, 0:128], encT_pad[:, 128:256]], axis=2)
        m["encTp"] = np.ascontiguousarray(encT_p.reshape(NB, 128, 2 * T)).astype(FP8)
        m["encN"] = enc_c.astype(BF16)
        dpk_c = dpk.copy()
        dpk_c[0:NB, DK_C0 : DK_C0 + H] = c0f[bs]
        m["dpk"] = dpk_c.astype(BF16)
        wpk_c = wpk.copy()
        qTc = q[bs].T  # [H, NB]
        wpk_c[:, PK_QT0 : PK_QT0 + NB] = qTc[0:128]
        wpk_c[0:72, PK_QT1 : PK_QT1 + NB] = qTc[128:200]
        m["wpk"] = wpk_c.astype(BF16)
        m["x0s"] = np.ascontiguousarray(x0[bs])
        in_maps.append(m)
    return in_maps


def kernel(**inputs):
    from concourse.bass_utils import run_bass_kernel_spmd

    if "nc" not in _CACHE:
        _CACHE["nc"] = _build_module()
    nc = _CACHE["nc"]

    in_maps = _prep_inputs(**inputs)
    res = run_bass_kernel_spmd(nc, in_maps, core_ids=list(range(NCORES)))
    # y2 per core: [NB, NSTEPS] -> full output [B, NSTEPS]
    out = np.concatenate([r["y2"] for r in res.results], axis=0)
    return np.ascontiguousarray(out.astype(np.float32))


# revision 17
# speedup vs baseline: 1.6877x; 1.0410x over previous
"""Trainium2 Bass kernel for nn_DecoderAttention (Bahdanau attention + LSTM decoder).

Data-parallel over batch: B=128 split across 8 NeuronCores (16 batches/core).
All FLOPs run on device; the host only reshuffles layouts (transpose / dtype
cast / fp8 DoubleRow packing / weight concat with bias rows folded in).

Per-core device pipeline (cost-model-aware layout):
  phase 0: ONE packed DMA for all small attention weights (+one fp32 qb DMA),
           qprojT = Wa @ q^T (+ ba + bua) on PE
  phase 1: per batch b: kprojT = Ua @ enc_b^T as fp8 DoubleRow matmuls
           (K=200 packed 2/partition, one pass, 0.5 cyc/row),
           tanh(kproj/64 + qprojT[:, b]) on ACT -> e tiles [h, t] bf16;
           scores as FLIPPED matmuls: e chunk stationary (K=h, M=128 t's of
           one stride-16 class), Va moving (N=1) -> scores accumulate into
           one [128, 256] PSUM tile, column 16*b+c;
           per WAVE of 4 batches: exp slice [128, 64] -> p (bf16,
           unnormalized), per-batch colsum matmuls -> zc, context via FLIPPED
           matmuls (encN chunk stationary K=t, p column moving N=1) -> ctxT
           accumulates [h, b] in PSUM. All of it hides inside phase 1.
  phase 2 (tail): Z totals via two tiny PE reductions, 1/Z broadcast via a
           K=1 outer-product matmul, ctxT scale on DVE, G0 closes an
           accumulation whose q-terms ran during phase 1.
  phase 3: 5 serial decoder steps, all-bf16 elementwise:
           gates = G0(PSUM) + x*wxr fused on DVE (scalar_tensor_tensor,
           split f,i,o vs g), gate order permuted so one sigmoid covers
           f,i,o; MLP in bf16; m3 flipped so x_next = out [16, 1] feeds the
           next step's scalar directly from PSUM.
"""

import numpy as np
import ml_dtypes

B, T, H = 128, 2048, 200
NCORES = 8
NB = B // NCORES  # 16 batches per core
NSTEPS = 5
G4 = 4 * H  # 800 gate width
NCH = T // 128  # 16 stride-class chunks (t = 16*k + c -> partition k, chunk c)

_CACHE = {}

BF16 = ml_dtypes.bfloat16
FP8 = ml_dtypes.float8_e4m3fn

# packed attention-weight tensor column layout (bf16, [128, PK_C])
PK_WA0, PK_WA1 = 0, 200          # wa0 [128,200] | wa1 [72,200]
PK_VA0, PK_VA1 = 400, 401        # va columns
PK_QT0, PK_QT1 = 402, 418        # qt [*,16]
PK_C = 434

# packed decoder-weight tensor column layout (bf16, [128, DK_C])
DK_WIHC0, DK_WIHC1 = 0, 800      # wihc0 [128,800] | wihc1 [73,800] (row72=bias)
DK_WHH0, DK_WHH1 = 1600, 2400    # whh [128|72, 800]
DK_WXR = 3200                    # wxr [16, 800]
DK_W1T0, DK_W1T1 = 4000, 4100    # w1t [128|73, 100] (row72=b1)
DK_W2T = 4200                    # w2t [101, 50] (row100=b2)
DK_W3T = 4250                    # w3t [51, 1] (row50=b3)
DK_C0 = 4251                     # c0 [16, 200]
DK_C = 4451


def _build_module():
    """Build the Bass module (same NEFF for all 8 cores)."""
    from contextlib import ExitStack

    import concourse.bass as bass
    import concourse.tile as tile
    from concourse import bacc, mybir
    from concourse.masks import make_identity

    dt = mybir.dt
    AF = mybir.ActivationFunctionType
    OP = mybir.AluOpType

    nc = bacc.Bacc(
        "TRN2",
        target_bir_lowering=False,
        debug=False,
        num_devices=NCORES,
    )

    # ---- DRAM tensors (per-core shards; weights replicated) ----
    d_encT = nc.dram_tensor(
        "encTp", [NB, 128, 2 * T], dt.float8e4, kind="ExternalInput"
    ).ap()  # DoubleRow packing: col i*T+t, partition p <-> h_in = i*128+p
    d_encN = nc.dram_tensor("encN", [NB, T, H], dt.bfloat16, kind="ExternalInput").ap()
    d_x0 = nc.dram_tensor("x0s", [NB, 1], dt.float32, kind="ExternalInput").ap()
    d_UaT = nc.dram_tensor("UaTp", [128, 2 * 208], dt.float8e4, kind="ExternalInput").ap()
    d_wpk = nc.dram_tensor("wpk", [128, PK_C], dt.bfloat16, kind="ExternalInput").ap()
    d_qb = nc.dram_tensor("qb2", [128, 2], dt.float32, kind="ExternalInput").ap()
    d_dpk = nc.dram_tensor("dpk", [128, DK_C], dt.bfloat16, kind="ExternalInput").ap()
    d_ones_b = nc.dram_tensor("ones_b", [1, NB], dt.bfloat16, kind="ExternalInput").ap()
    d_y = nc.dram_tensor("y2", [NB, NSTEPS], dt.float32, kind="ExternalOutput").ap()

    H0, H1 = 128, H - 128  # 128 + 72 partition chunks of the hidden dim

    with tile.TileContext(nc) as tc, ExitStack() as ctx:
        # ---------- persistent pools ----------
        wpool = ctx.enter_context(tc.tile_pool(name="weights", bufs=1))
        spool = ctx.enter_context(tc.tile_pool(name="smalls", bufs=1))

        # warmup: preload the tanh/exp ACT table set while DMAs stream
        wt_a = spool.tile([1, 8], dt.float32)
        nc.vector.memset(wt_a[:], 0.0)
        wt_b = spool.tile([1, 8], dt.float32)
        nc.scalar.activation(wt_b[:], wt_a[:], AF.Tanh)

        # identity for the decoder's h transposes (bf16)
        id_bf = wpool.tile([128, 128], dt.bfloat16)
        make_identity(nc, id_bf[:])

        # ones columns/rows for the tiny PE reductions (sliced on read)
        ones_c_bf = wpool.tile([128, 1], dt.bfloat16)
        nc.vector.memset(ones_c_bf[:], 1.0)
        ones_c_f = wpool.tile([128, 1], dt.float32)
        nc.vector.memset(ones_c_f[:], 1.0)
        ones_sq_f = wpool.tile([NCH, 128], dt.float32)
        nc.vector.memset(ones_sq_f[:], 1.0)

        # packed attention weights: one DMA instead of ten
        wpk = wpool.tile([128, PK_C], dt.bfloat16)
        nc.scalar.dma_start(wpk[:], d_wpk[:, :])
        qb2 = wpool.tile([128, 2], dt.float32)
        nc.scalar.dma_start(qb2[:], d_qb[:, :])
        wa0 = wpk[:, PK_WA0 : PK_WA0 + 200]
        wa1 = wpk[0:H1, PK_WA1 : PK_WA1 + 200]
        va0 = wpk[:, PK_VA0 : PK_VA0 + 1]
        va1 = wpk[0:H1, PK_VA1 : PK_VA1 + 1]
        qt0 = wpk[:, PK_QT0 : PK_QT0 + NB]
        qt1 = wpk[0:H1, PK_QT1 : PK_QT1 + NB]

        ua_p = wpool.tile([128, 2 * 208], dt.float8e4)
        ua3 = ua_p[:].rearrange("p (i m) -> p i m", i=2)

        # packed decoder weights: one DMA (deferred below, behind first encT)
        dpk = wpool.tile([128, DK_C], dt.bfloat16)
        wihc0 = dpk[:, DK_WIHC0 : DK_WIHC0 + G4]
        wihc1 = dpk[0 : H1 + 1, DK_WIHC1 : DK_WIHC1 + G4]
        whh0 = dpk[:, DK_WHH0 : DK_WHH0 + G4]
        whh1 = dpk[0:H1, DK_WHH1 : DK_WHH1 + G4]
        wxr_sb = dpk[0:NB, DK_WXR : DK_WXR + G4]
        w1t0 = dpk[:, DK_W1T0 : DK_W1T0 + 100]
        w1t1 = dpk[0 : H1 + 1, DK_W1T1 : DK_W1T1 + 100]
        w2t = dpk[0:101, DK_W2T : DK_W2T + 50]
        w3t = dpk[0:51, DK_W3T : DK_W3T + 1]
        c0_sb = dpk[0:NB, DK_C0 : DK_C0 + H]

        # ---------- phase 0: qprojT = Wa @ q^T + (ba + bua) ----------
        qproj0 = spool.tile([H0, NB], dt.float32)  # fp32 bias tiles for tanh
        qproj1 = spool.tile([H1, NB], dt.float32)
        with tc.tile_pool(name="qp_psum", bufs=1, space="PSUM") as qp_ps:
            for m, (msz, qdst) in enumerate([(H0, qproj0), (H1, qproj1)]):
                mlo = m * H0
                ps = qp_ps.tile([128, NB], dt.float32, tag="qp")
                nc.tensor.matmul(
                    ps[0:msz, :], wa0[:, mlo : mlo + msz], qt0, start=True, stop=False
                )
                nc.tensor.matmul(
                    ps[0:msz, :], wa1[:, mlo : mlo + msz], qt1, start=False, stop=True
                )
                nc.vector.tensor_scalar_add(
                    qdst[:], ps[0:msz, :], qb2[0:msz, m : m + 1]
                )

        # G0 PSUM lives from phase 2 through the decoder; allocate its pool
        # ahead of the attention pools so releases stay LIFO
        g0_pool = ctx.enter_context(tc.tile_pool(name="g0_psum", bufs=1, space="PSUM"))
        gp = g0_pool.tile([NB, G4], dt.float32, tag="g0")

        # ---------- phase 1: kproj + tanh + scores + per-wave softmax/ctx ----
        encT_pool = ctx.enter_context(tc.tile_pool(name="encT_pool", bufs=3))
        e_pool = ctx.enter_context(tc.tile_pool(name="e_pool", bufs=4))
        encN_pool = ctx.enter_context(tc.tile_pool(name="encN_pool", bufs=16))
        from contextlib import ExitStack as _ES
        att_ctx = _ES()
        sc_pool = att_ctx.enter_context(tc.tile_pool(name="sc_psum", bufs=1, space="PSUM"))
        ct_pool = att_ctx.enter_context(tc.tile_pool(name="ct_psum", bufs=1, space="PSUM"))
        sc = sc_pool.tile([128, NB * NCH + NB], dt.float32, tag="sc")
        p_sb = spool.tile([128, NB * NCH], dt.bfloat16)
        zc = sc[0:NCH, NB * NCH : NB * NCH + NB]
        # one PSUM tile (PSUM tiles are bank-granular): ct0 | ct1 columns
        ctz = ct_pool.tile([128, 2 * NB], dt.float32, tag="ctz")
        ct0_ps = ctz[:, 0:NB]
        ct1_ps = ctz[0:H1, NB : 2 * NB]

        en_tiles = []
        kp_hist = []  # last kproj matmul of each batch (encN pacing anchor)
        e_saved = []  # (e0, e1) per batch; scores emitted with 1-batch skew

        def emit_scores(b):
            e0, e1 = e_saved[b]
            for c in range(NCH):
                col = b * NCH + c
                nc.tensor.matmul(
                    sc[:, col : col + 1], e0[:, c : T : NCH], va0,
                    start=True, stop=False,
                )
                nc.tensor.matmul(
                    sc[:, col : col + 1], e1[:, c : T : NCH], va1,
                    start=False, stop=True,
                )

        def emit_wave(w):
            # exp + Z colsums + context for batches 4w..4w+3 (scores ready)
            lo = 4 * w * NCH
            nc.scalar.activation(
                p_sb[:, lo : lo + 4 * NCH], sc[:, lo : lo + 4 * NCH], AF.Exp
            )
            for b in range(4 * w, 4 * w + 4):
                nc.tensor.matmul(
                    zc[:, b : b + 1],
                    p_sb[:, b * NCH : (b + 1) * NCH],
                    ones_c_bf[:],
                    start=True,
                    stop=True,
                )
                for c in range(NCH):
                    pcol = p_sb[:, b * NCH + c : b * NCH + c + 1]
                    nc.tensor.matmul(
                        ct0_ps[:, b : b + 1],
                        en_tiles[b][:, c * H : c * H + H0],
                        pcol,
                        start=(c == 0),
                        stop=(c == NCH - 1),
                    )
                    nc.tensor.matmul(
                        ct1_ps[:, b : b + 1],
                        en_tiles[b][:, c * H + H0 : (c + 1) * H],
                        pcol,
                        start=(c == 0),
                        stop=(c == NCH - 1),
                    )

        with tc.tile_pool(name="kp_psum", bufs=2, space="PSUM") as kp_ps:
            for b in range(NB):
                et = encT_pool.tile([128, 2 * T], dt.float8e4, tag="et")
                nc.sync.dma_start(et[:], d_encT[b, :])
                et3 = et[:].rearrange("p (i t) -> p i t", i=2)
                if b == 0:
                    nc.sync.dma_start(ua_p[:], d_UaT[:, :])
                if b == 3:
                    # single packed decoder-weight DMA, early but off the
                    # startup critical path
                    nc.sync.dma_start(dpk[:], d_dpk[:, :])
                e0 = e_pool.tile([H0, T], dt.bfloat16, tag="e0")
                e1 = e_pool.tile([H1, T], dt.bfloat16, tag="e1")
                i_kp = None
                for m, (msz, edst, qp) in enumerate(
                    [(H0, e0, qproj0), (H1, e1, qproj1)]
                ):
                    mlo = m * H0
                    for th in range(2):  # 1024-wide psum tiles
                        ps = kp_ps.tile([128, 1024], dt.float32, tag="kp")
                        for n in range(2):
                            c0c = th * 1024 + n * 512
                            i_kp = nc.tensor.matmul(
                                ps[0:msz, n * 512 : (n + 1) * 512],
                                ua3[:, :, mlo : mlo + msz],
                                et3[:, :, c0c : c0c + 512],
                                start=True,
                                stop=True,
                                perf_mode=mybir.MatmulPerfMode.DoubleRow,
                            )
                        # e = tanh(kproj/64 + qproj[:, b]) ; write bf16
                        nc.scalar.activation(
                            edst[:, th * 1024 : (th + 1) * 1024],
                            ps[0:msz, :],
                            AF.Tanh,
                            bias=qp[:, b : b + 1],
                            scale=1.0 / 64.0,
                        )
                # scores with a 1-batch skew; waves fire as they complete
                if b > 0:
                    emit_scores(b - 1)
                    if b % 4 == 0:
                        emit_wave(b // 4 - 1)
                e_saved.append((e0, e1))
                kp_hist.append(i_kp)
                # encN loads paced on the (otherwise idle) SWDGE ring, gated
                # two batches back so the tail tiles land before their ctx
                import bass_rust as _br

                en = encN_pool.tile(
                    [128, NCH * H], dt.bfloat16, name=f"en{b}", tag="en"
                )
                i_en = nc.gpsimd.dma_start(
                    en[:],
                    d_encN[b].rearrange("(p n) h -> p (n h)", p=128),
                )
                _br.add_dep_helper(
                    i_en.ins, kp_hist[max(0, b - 2)].ins, sync=True,
                    reason="encN paced behind kproj two batches back",
                )
                en_tiles.append(en)
            emit_scores(NB - 1)
            emit_wave(3)

        # ---------- phase 2: Z totals, 1/Z, ctx scale, G0 ----------
        ct0 = spool.tile([H0, NB], dt.bfloat16)
        ct1 = spool.tile([H1 + 1, NB], dt.bfloat16)  # row 72 = ones (bias row)
        nc.scalar.dma_start(ct1[H1 : H1 + 1, :], d_ones_b[:, :])

        with tc.tile_pool(name="z_psum", bufs=1, space="PSUM") as z_ps:
            # q-dependent G0 terms: no ctx dependency, run right away
            for n, nsz in [(0, 512), (512, G4 - 512)]:
                nc.tensor.matmul(
                    gp[:, n : n + nsz], qt0, whh0[:, n : n + nsz],
                    start=True, stop=False,
                )
                nc.tensor.matmul(
                    gp[:, n : n + nsz], qt1, whh1[:, n : n + nsz],
                    start=False, stop=False,
                )
            # Z per batch broadcast down all 128 partitions in one matmul
            # (lhsT = ones [16, 128] -> out[r, b] = sum_ch zc[ch, b]), then
            # reciprocal straight into SBUF
            zc_sb = spool.tile([NCH, NB], dt.float32)
            nc.vector.tensor_copy(zc_sb[:], zc[:])
            zbc = z_ps.tile([128, NB], dt.float32, tag="zbc")
            nc.tensor.matmul(zbc[:], ones_sq_f[:], zc_sb[:], start=True, stop=True)
            rzb_sb = spool.tile([128, NB], dt.float32)
            nc.vector.reciprocal(rzb_sb[:], zbc[:])
            # normalize: ctxT = ctx_raw * (1/Z) broadcast, cast bf16
            nc.vector.tensor_tensor(ct0[:], ct0_ps[:], rzb_sb[:], op=OP.mult)
            nc.vector.tensor_tensor(
                ct1[0:H1, :], ct1_ps[:], rzb_sb[0:H1, :], op=OP.mult
            )
            for n, nsz in [(0, 512), (512, G4 - 512)]:
                nc.tensor.matmul(
                    gp[:, n : n + nsz], ct0[:], wihc0[:, n : n + nsz],
                    start=False, stop=False,
                )
                nc.tensor.matmul(
                    gp[:, n : n + nsz], ct1, wihc1[:, n : n + nsz],
                    start=False, stop=True,
                )
        att_ctx.close()  # release sc/ct PSUM banks before the decoder

        # ---------- phase 3: decoder steps (all bf16, gate order f,i,o|g) ---
        x_sb = spool.tile([NB, 1], dt.float32)
        nc.sync.dma_start(x_sb[:], d_x0[:, :])
        xn_all = spool.tile([NB, NSTEPS], dt.float32)
        ht0 = spool.tile([H0, NB], dt.bfloat16)
        ht1 = spool.tile([H1 + 1, NB], dt.bfloat16)  # row 72 = ones (b1 row)
        nc.scalar.dma_start(ht1[H1 : H1 + 1, :], d_ones_b[:, :])
        o1t = spool.tile([101, NB], dt.bfloat16)  # row 100 = ones (b2 row)
        nc.scalar.dma_start(o1t[100:101, :], d_ones_b[:, :])
        o2t = spool.tile([51, NB], dt.bfloat16)  # row 50 = ones (b3 row)
        nc.scalar.dma_start(o2t[50:51, :], d_ones_b[:, :])

        with (
            tc.tile_pool(name="ls", bufs=2) as ls,
            tc.tile_pool(name="ls_psum", bufs=3, space="PSUM") as lp,
            tc.tile_pool(name="m3_psum", bufs=2, space="PSUM") as mp,
        ):
            xt = x_sb
            for t in range(NSTEPS):
                # gates = g0 + x * wxr, fused on DVE; split f,i,o vs g so
                # the sigmoid can start before the g slice is computed
                gates = ls.tile([NB, G4], dt.bfloat16, tag="gates")
                for glo, ghi in ((0, 2 * H), (3 * H, 4 * H), (2 * H, 3 * H)):
                    nc.vector.scalar_tensor_tensor(
                        gates[:, glo:ghi], wxr_sb[:, glo:ghi], xt[:, 0:1],
                        gp[:, glo:ghi], op0=OP.mult, op1=OP.add,
                    )
                # f,i sigmoid first (feeds t1/t2), then g tanh, then o
                sfio = ls.tile([NB, 3 * H], dt.bfloat16, tag="sfio")
                nc.scalar.activation(sfio[:, 0 : 2 * H], gates[:, 0 : 2 * H], AF.Sigmoid)
                tg = ls.tile([NB, H], dt.bfloat16, tag="tg")
                nc.scalar.activation(tg[:], gates[:, 3 * H : 4 * H], AF.Tanh)
                nc.scalar.activation(
                    sfio[:, 2 * H : 3 * H], gates[:, 2 * H : 3 * H], AF.Sigmoid
                )
                t1 = ls.tile([NB, H], dt.bfloat16, tag="t1")
                nc.vector.tensor_tensor(t1[:], sfio[:, 0:H], c0_sb, op=OP.mult)
                t2 = ls.tile([NB, H], dt.bfloat16, tag="t2")
                nc.vector.tensor_tensor(t2[:], sfio[:, H : 2 * H], tg[:], op=OP.mult)
                cn = ls.tile([NB, H], dt.bfloat16, tag="cn")
                nc.vector.tensor_tensor(cn[:], t1[:], t2[:], op=OP.add)
                tcn = ls.tile([NB, H], dt.bfloat16, tag="tcn")
                nc.scalar.activation(tcn[:], cn[:], AF.Tanh)
                hh = ls.tile([NB, H], dt.bfloat16, tag="hh")
                nc.vector.tensor_tensor(hh[:], sfio[:, 2 * H : 3 * H], tcn[:], op=OP.mult)
                # transpose h -> ht0/ht1 (feature-major for the MLP); relu
                # folded into the PSUM->SBUF copies (DVE max / ACT relu)
                tp0 = lp.tile([128, NB], dt.bfloat16, tag="lsps")
                nc.tensor.transpose(tp0[:], hh[:, 0:H0], id_bf[0:NB, 0:NB])
                nc.vector.tensor_scalar_max(ht0[:], tp0[:], 0.0)
                tp1 = lp.tile([128, NB], dt.bfloat16, tag="lsps")
                nc.tensor.transpose(tp1[0:H1, :], hh[:, H0:H], id_bf[0:NB, 0:NB])
                nc.scalar.activation(ht1[0:H1, :], tp1[0:H1, :], AF.Relu)
                # MLP in feature-major, biases via ones rows
                m1 = lp.tile([100, NB], dt.float32, tag="lsps")
                nc.tensor.matmul(m1[:], w1t0, ht0[:], start=True, stop=False)
                nc.tensor.matmul(m1[:], w1t1, ht1[:], start=False, stop=True)
                nc.vector.tensor_scalar_max(o1t[0:100, :], m1[:], 0.0)
                m2 = lp.tile([50, NB], dt.float32, tag="lsps")
                nc.tensor.matmul(m2[:], w2t, o1t[:], start=True, stop=True)
                nc.vector.tensor_scalar_max(o2t[0:50, :], m2[:], 0.0)
                # m3 flipped: o2 stationary, w3 moving -> out [NB, 1] is
                # directly the next step's x (read from PSUM as STT scalar)
                m3 = mp.tile([NB, 1], dt.float32, tag="m3")
                nc.tensor.matmul(m3[:], o2t[:], w3t, start=True, stop=True)
                nc.vector.tensor_copy(xn_all[:, t : t + 1], m3[:])
                xt = m3
            nc.sync.dma_start(d_y[:, :], xn_all[:])

    # Bacc lowering: register allocation + wait splitting (<=1 wait/inst on HW)
    nc.compile()
    return nc


def _prep_inputs(x, h0, c0, encoder_output, Wa, ba, Ua, bua, Va, bva,
                 W_ih, W_hh, b_ih, b_hh, W1, b1, W2, b2, W3, b3):
    """Host-side layout prep -> list of per-core input maps."""
    f32 = np.float32
    enc = np.ascontiguousarray(encoder_output, dtype=f32)
    q = np.asarray(h0, dtype=f32)[0]          # [B, H]
    c0f = np.asarray(c0, dtype=f32)[0]        # [B, H]
    x0 = np.asarray(x, dtype=f32).reshape(B, 1)

    # gate permutation: torch order (i,f,g,o) -> device order (f,i,o,g) so
    # one sigmoid instr covers f,i,o and tanh covers g
    gperm = np.concatenate(
        [np.arange(H, 2 * H), np.arange(0, H), np.arange(3 * H, 4 * H),
         np.arange(2 * H, 3 * H)]
    )
    W_ihp = np.asarray(W_ih, f32)[gperm]
    W_hhp = np.asarray(W_hh, f32)[gperm]
    bp = (np.asarray(b_ih, f32) + np.asarray(b_hh, f32))[gperm]

    # Ua scaled x64 into fp8 comfortable range; kernel rescales inside tanh.
    # DoubleRow packing: partition p holds h_in = p (i=0) and 128+p (i=1);
    # M padded 200->208 so the pair stride is 16B-aligned.
    uaT = np.asarray(Ua, f32).T * 64.0  # [h_in, h_out]
    uaT_pad = np.zeros((256, 208), f32)
    uaT_pad[0:H, 0:H] = uaT
    uaT_p = np.stack([uaT_pad[0:128], uaT_pad[128:256]], axis=1)  # [128, 2, 208]

    # packed attention weights
    wpk = np.zeros((128, PK_C), f32)
    waT = np.asarray(Wa, f32).T  # [h_in, h_out]
    wpk[:, PK_WA0 : PK_WA0 + 200] = waT[0:128]
    wpk[0:72, PK_WA1 : PK_WA1 + 200] = waT[128:200]
    va = np.asarray(Va, f32)[0]
    wpk[:, PK_VA0] = va[0:128]
    wpk[0:72, PK_VA1] = va[128:200]
    qb = np.asarray(ba, f32) + np.asarray(bua, f32)
    qb2 = np.zeros((128, 2), f32)
    qb2[:, 0] = qb[0:128]
    qb2[0:72, 1] = qb[128:200]

    # packed decoder weights
    dpk = np.zeros((128, DK_C), f32)
    w_ihcT = W_ihp[:, 1:].T  # [H, G4]
    dpk[:, DK_WIHC0 : DK_WIHC0 + G4] = w_ihcT[0:128]
    dpk[0:72, DK_WIHC1 : DK_WIHC1 + G4] = w_ihcT[128:200]
    dpk[72, DK_WIHC1 : DK_WIHC1 + G4] = bp
    w_hhT = W_hhp.T
    dpk[:, DK_WHH0 : DK_WHH0 + G4] = w_hhT[0:128]
    dpk[0:72, DK_WHH1 : DK_WHH1 + G4] = w_hhT[128:200]
    dpk[0:NB, DK_WXR : DK_WXR + G4] = np.broadcast_to(
        W_ihp[:, 0].reshape(1, G4), (NB, G4)
    )
    w1T = np.asarray(W1, f32).T
    dpk[:, DK_W1T0 : DK_W1T0 + 100] = w1T[0:128]
    dpk[0:72, DK_W1T1 : DK_W1T1 + 100] = w1T[128:200]
    dpk[72, DK_W1T1 : DK_W1T1 + 100] = np.asarray(b1, f32)
    dpk[0:100, DK_W2T : DK_W2T + 50] = np.asarray(W2, f32).T
    dpk[100, DK_W2T : DK_W2T + 50] = np.asarray(b2, f32)
    dpk[0:50, DK_W3T] = np.asarray(W3, f32)[0]
    dpk[50, DK_W3T] = np.asarray(b3, f32)[0]

    shared = {
        "UaTp": np.ascontiguousarray(uaT_p.reshape(128, 2 * 208)).astype(FP8),
        "qb2": qb2,
        "ones_b": np.ones((1, NB), BF16),
    }

    in_maps = []
    for cix in range(NCORES):
        bs = slice(cix * NB, (cix + 1) * NB)
        enc_c = enc[bs]  # [NB, T, H]
        m = dict(shared)
        encT_c = enc_c.transpose(0, 2, 1)  # [NB, H, T]
        encT_pad = np.concatenate(
            [encT_c, np.zeros((NB, 56, T), f32)], axis=1
        )  # [NB, 256, T]
        encT_p = np.stack([encT_pad[:, 0:128], encT_pad[:, 128:256]], axis=2)
        m["encTp"] = np.ascontiguousarray(encT_p.reshape(NB, 128, 2 * T)).astype(FP8)
        m["encN"] = enc_c.astype(BF16)
        dpk_c = dpk.copy()
        dpk_c[0:NB, DK_C0 : DK_C0 + H] = c0f[bs]
        m["dpk"] = dpk_c.astype(BF16)
        wpk_c = wpk.copy()
        qTc = q[bs].T  # [H, NB]
        wpk_c[:, PK_QT0 : PK_QT0 + NB] = qTc[0:128]
        wpk_c[0:72, PK_QT1 : PK_QT1 + NB] = qTc[128:200]
        m["wpk"] = wpk_c.astype(BF16)
        m["x0s"] = np.ascontiguousarray(x0[bs])
        in_maps.append(m)
    return in_maps


def kernel(**inputs):
    from concourse.bass_utils import run_bass_kernel_spmd

    if "nc" not in _CACHE:
        _CACHE["nc"] = _build_module()
    nc = _CACHE["nc"]

    in_maps = _prep_inputs(**inputs)
    res = run_bass_kernel_spmd(nc, in_maps, core_ids=list(range(NCORES)))
    # y2 per core: [NB, NSTEPS] -> full output [B, NSTEPS]
    out = np.concatenate([r["y2"] for r in res.results], axis=0)
    return np.ascontiguousarray(out.astype(np.float32))
